# revision 1
# baseline (speedup 1.0000x reference)
"""RWKV6 (x060) block kernel for Trainium2 across 8 NeuronCores.

Sharding: DP2 x TP4.  Cores 0-3 compute batch 0, cores 4-7 batch 1.
Within each group of 4 cores: heads (H/4 per core), projection output
channels (C/4) and the FFN dim (FFN/4) are tensor-parallel.  Collectives
per group: AllReduce of the attention output-projection partials (needed
in full for LN2 + FFN contraction), a ReduceScatter of the same partials
(gives each core its residual slice), and a ReduceScatter of the FFN
value-projection partials.

Layout: channel-major activations (channels on SBUF partitions, time on
the free dim).  Weights are pre-transposed / pre-tiled on the host so
every weight DMA is a large contiguous transfer.  Big matmuls run in
fp16 (fp32 PSUM accumulate), LN statistics matmuls in fp32r, and the
WKV6 recurrence in fp32 using a chunked linear-attention form (L=128):

  per chunk:  Lam = cumsum(-exp(w))              (tensor_tensor_scan)
              R~ = r*exp(Lam_{t-1}), K~ = k*exp(-Lam), K^ = k*exp(Lam_L-Lam)
              A^T = K~ @ R~^T  (strict upper mask, s<t)
              O   = A'^T.T @ V + R~^T.T @ S0 + (r.u.k)_t * v_t
              S'  = exp(Lam_L) * S0 + K^^T @ V
"""

import contextlib
import functools
import numpy as np

import concourse.bacc as bacc
import concourse.bass as bass
import concourse.mybir as mybir
import concourse.tile as tile

F32 = mybir.dt.float32
F32R = mybir.dt.float32r
F16 = mybir.dt.float16
AX = mybir.AxisListType
OP = mybir.AluOpType
AF = mybir.ActivationFunctionType

P = 128
EPS_LN = 1e-5
EPS_GN = 64e-5


class CFG:
    def __init__(self, b=2, t=1024, c=2048, hs=64, h=32, ffn=7168, tm=32, td=64,
                 tpg=4, l=128, tc=256, tc2=512):
        self.b, self.t, self.c, self.hs, self.h = b, t, c, hs, h
        self.ffn, self.tm, self.td = ffn, tm, td
        self.tpg = tpg
        self.l = l
        self.tc = tc
        self.tc2 = tc2
        self.n_cores = 2 * tpg
        self.ct = c // P
        self.nh = h // tpg
        self.npair = self.nh // 2
        self.oc = c // tpg
        self.oct = self.oc // P
        self.fc = ffn // tpg
        self.fct = self.fc // P
        self.nsc = t // tc
        self.nsc2 = t // tc2
        self.nl = tc // l
        assert self.nh % 2 == 0 and hs == 64 and self.oc % P == 0
        assert self.fc % P == 0 and t % tc == 0 and tc % l == 0 and t % tc2 == 0


FULL = CFG()


# ---------------------------------------------------------------------------
# builder
# ---------------------------------------------------------------------------

def build_nc(g: CFG, debug=False, phases=4):
    nc = bacc.Bacc("TRN2", target_bir_lowering=False, num_devices=g.n_cores)

    def din(name, shape, dt=F16):
        return nc.dram_tensor(name, list(shape), dt, kind="ExternalInput")

    tn = {}
    tn["xT"] = din("xT", (g.c, g.t), F32)
    tn["x_sl"] = din("x_sl", (g.oc, g.t), F32)          # local rows of x (batch)
    # lhsT weight banks: layout (outer, 128 kin, inner, 128 m) contiguous
    tn["wrT"] = din("wrT", (g.oct, P, g.ct, P))
    tn["wkT"] = din("wkT", (g.oct, P, g.ct, P))
    tn["woT"] = din("woT", (g.ct, P, g.oct, P))
    tn["wkfT"] = din("wkfT", (g.fct, P, g.ct, P))
    tn["wvfT"] = din("wvfT", (g.ct, P, g.fct, P))
    tn["wrfT"] = din("wrfT", (g.oct, P, g.ct, P))
    # rhs weight banks (kin-major): (128 kin, ct, oc)
    tn["wvT"] = din("wvT", (P, g.ct, g.oc))
    tn["wgT"] = din("wgT", (P, g.ct, g.oc))
    tn["tdw1"] = din("tdw1", (P, g.ct, g.td))
    tn["tdw2"] = din("tdw2", (g.td, g.oc))
    tn["mw1"] = din("mw1", (P, g.ct, 5 * g.tm))
    tn["mw2a"] = din("mw2a", (4 * g.tm, g.c))
    tn["mw2b"] = din("mw2b", (g.tm, g.c))
    # coef rows: 0 maa_x, 1..5 maa w/k/v/r/g, 6 ffn_k, 7 ffn_r, 8 ln1_w, 9 ln2_w
    tn["coef"] = din("coef", (P, 10, g.ct), F32)
    tn["tdec"] = din("tdec", (P, g.oct), F32)
    tn["u"] = din("u", (P, g.npair), F32)
    tn["mask_su"] = din("mask_su", (g.l, g.l), F32)
    tn["ident"] = din("ident", (P, P), F16)
    tn["ones_r"] = din("ones_r", (P, 1), F32)

    tn["cc1_in"] = nc.dram_tensor("cc1_in", [g.c, g.t], F32)
    tn["cc1_out"] = nc.dram_tensor("cc1_out", [g.c, g.t], F32)
    tn["cc1b_out"] = nc.dram_tensor("cc1b_out", [g.oc, g.t], F32)
    tn["cc2_in"] = nc.dram_tensor("cc2_in", [g.c, g.t], F32)
    tn["cc2_out"] = nc.dram_tensor("cc2_out", [g.oc, g.t], F32)
    tn["stat_d"] = nc.dram_tensor("stat_d", [4, g.t], F32)
    tn["yT"] = nc.dram_tensor("yT", [g.oc, g.t], F32, kind="ExternalOutput")
    tn["groups"] = [list(range(g.tpg)), list(range(g.tpg, 2 * g.tpg))]

    if debug:
        for nm, shape in [("dbg_vark", (g.c, g.tc)), ("dbg_r", (g.oc, g.tc)),
                          ("dbg_k", (g.oc, g.tc)), ("dbg_w", (g.oc, g.tc)),
                          ("dbg_v", (g.tc, g.oc)), ("dbg_g", (g.tc, g.oc)),
                          ("dbg_o", (g.t, g.oc)), ("dbg_og", (g.l, g.oc)),
                          ("dbg_xn", (g.c, g.t)), ("dbg_xx", (g.c, g.t)),
                          ("dbg_ogT", (g.oc, g.tc)),
                          ("dbg_S", (P, 4 * 64)), ("dbg_ktm", (P, P)),
                          ("dbg_lam", (P, 4 * 64))]:
            tn[nm] = nc.dram_tensor(nm, list(shape), F32)
    tn["debug"] = debug
    tn["phases"] = phases
    with tile.TileContext(nc) as tc:
        with contextlib.ExitStack() as ctx:
            _body(ctx, nc, tc, g, tn)
    return nc


def _body(ctx, nc, tc, g, tn):
    dma = nc.sync.dma_start

    def pool(name, bufs, space="SBUF"):
        return ctx.enter_context(tc.tile_pool(name=name, bufs=bufs, space=space))

    # ---------------- global resident constants ----------------
    cp = pool("consts", 1)
    c_coef = cp.tile([P, 10, g.ct], F32)
    dma(out=c_coef[:], in_=tn["coef"][:, :, :])
    c_tdec = cp.tile([P, g.oct], F32)
    dma(out=c_tdec[:], in_=tn["tdec"][:, :])
    c_u = cp.tile([P, g.npair], F32)
    dma(out=c_u[:], in_=tn["u"][:, :])
    c_mask = cp.tile([g.l, g.l], F32)
    dma(out=c_mask[:], in_=tn["mask_su"][:, :])
    c_id = cp.tile([P, P], F16)
    dma(out=c_id[:], in_=tn["ident"][:, :])
    c_id32 = cp.tile([P, P], F32)
    nc.vector.tensor_copy(out=c_id32[:], in_=c_id[:])
    c_ones_r = cp.tile([P, 1], F32R)
    dma(out=c_ones_r[:], in_=tn["ones_r"][:, :].bitcast(F32R))
    c_ones_f = cp.tile([P, 1], F32)
    dma(out=c_ones_f[:], in_=tn["ones_r"][:, :])
    c_neg1 = cp.tile([P, g.l], F32)
    nc.vector.memset(c_neg1, -1.0)
    c_eps = cp.tile([P, 1], F32)
    nc.vector.memset(c_eps, EPS_GN)
    c_eps1 = cp.tile([1, 1], F32)
    nc.vector.memset(c_eps1, EPS_LN)

    def coef_col(idx, j):
        return c_coef[:, idx, j:j + 1]

    sb = pool("statb", 1)
    b_mu = sb.tile([P, g.t], F32)
    b_rho = sb.tile([P, g.t], F32)

    xnp = pool("xn", 1)
    xn = xnp.tile([P, g.ct, g.t], F16)
    xx = xnp.tile([P, g.ct, g.t], F16)

    pp_proj = pool("pp_proj", 2, space="PSUM")

    NT5 = min(512, g.t)

    # ------------------------------------------------------------------
    # streaming LN helpers.  get_chunk(pool, j, csl) -> (P, len) f32 AP of
    # the pre-norm tensor rows tile j, time-slice csl.
    # ------------------------------------------------------------------
    def ln_stats(get_chunk, sqp, rowp, psp, stat_row, nrm):
        srow0 = rowp.tile([1, g.t], F32, tag="rows")
        srow1 = rowp.tile([1, g.t], F32, tag="rows")
        for ch in range(g.t // NT5):
            ps0 = psp.tile([1, NT5], F32, tag="st")
            ps1 = psp.tile([1, NT5], F32, tag="st")
            for j in range(g.ct):
                xt = get_chunk(sqp, j, bass.ts(ch, NT5))
                sq = sqp.tile([P, NT5], F32R, tag="sq")
                nc.scalar.square(out=sq[:], in_=xt)
                nc.tensor.matmul(ps0[:], c_ones_r, xt.bitcast(F32R),
                                 start=(j == 0), stop=(j == g.ct - 1))
                nc.tensor.matmul(ps1[:], c_ones_r, sq[:],
                                 start=(j == 0), stop=(j == g.ct - 1))
            nc.vector.tensor_copy(out=srow0[0:1, bass.ts(ch, NT5)], in_=ps0[:])
            nc.vector.tensor_copy(out=srow1[0:1, bass.ts(ch, NT5)], in_=ps1[:])
        mu = rowp.tile([1, g.t], F32, tag="rows")
        nc.scalar.mul(out=mu[:], in_=srow0[:], mul=1.0 / nrm)
        # reuse srow0 as musq scratch
        nc.vector.tensor_mul(out=srow0[:], in0=mu[:], in1=mu[:])
        nc.scalar.mul(out=srow1[:], in_=srow1[:], mul=1.0 / nrm)
        nc.vector.tensor_sub(out=srow1[:], in0=srow1[:], in1=srow0[:])
        nc.scalar.activation(out=srow1[:], in_=srow1[:], func=AF.Sqrt,
                             bias=c_eps1[:], scale=1.0)
        nc.vector.reciprocal(out=srow1[:], in_=srow1[:])
        dma(out=tn["stat_d"][stat_row:stat_row + 1, :], in_=mu[:])
        dma(out=tn["stat_d"][stat_row + 1:stat_row + 2, :], in_=srow1[:])

    def bcast_stats(stat_row):
        dma(out=b_mu[:], in_=bass.AP(tensor=tn["stat_d"], offset=stat_row * g.t,
                                     ap=[[0, P], [1, g.t]]))
        dma(out=b_rho[:], in_=bass.AP(tensor=tn["stat_d"],
                                      offset=(stat_row + 1) * g.t,
                                      ap=[[0, P], [1, g.t]]))

    def ln_apply(get_row, sqp, xn_t, xx_t, w_row):
        for j in range(g.ct):
            row = get_row(sqp, j)                      # (P, t) f32
            tsc = sqp.tile([P, g.t], F32, tag="lnt")
            eng = nc.gpsimd if j % 2 == 0 else nc.vector
            eng.tensor_sub(out=tsc[:], in0=row, in1=b_mu[:])
            nc.vector.scalar_tensor_tensor(out=xn_t[:, j, :], in0=tsc[:],
                                     scalar=coef_col(w_row, j), in1=b_rho[:],
                                     op0=OP.mult, op1=OP.mult)
            nc.vector.tensor_sub(out=xx_t[:, j, 1:g.t],
                                 in0=xn_t[:, j, 0:g.t - 1],
                                 in1=xn_t[:, j, 1:g.t])
            nc.scalar.mul(out=xx_t[:, j, 0:1], in_=xn_t[:, j, 0:1], mul=-1.0)

    # ==================================================================
    # phase 1a/1b: LN1 -> xn, xx  (x streamed from DRAM)
    # ==================================================================
    with tc.tile_pool(name="sq1", bufs=2) as sq1, \
         tc.tile_pool(name="row1", bufs=3) as row1, \
         tc.tile_pool(name="pps1", bufs=2, space="PSUM") as pps1:

        def x_chunk(sqp, j, csl):
            t_ = sqp.tile([P, NT5], F32R, tag="ldc")
            dma(out=t_[:], in_=tn["xT"][bass.ts(j, P), csl].bitcast(F32R))
            return t_[:]

        def x_row(sqp, j):
            t_ = sqp.tile([P, g.t], F32, tag="ldr")
            dma(out=t_[:], in_=tn["xT"][bass.ts(j, P), :])
            return t_[:]

        ln_stats(x_chunk, sq1, row1, pps1, 0, g.c)
        bcast_stats(0)
        ln_apply(x_row, sq1, xn, xx, 8)

    if tn["phases"] == 1:
        for o in range(g.oct):
            tmp = ctx.enter_context(tc.tile_pool(name="fin1", bufs=2))
            tt_ = tmp.tile([P, g.t], F32, tag="f")
            nc.vector.tensor_copy(out=tt_[:], in_=xn[:, o, :])
            dma(out=tn["yT"][bass.ts(o, P), :], in_=tt_[:])
        return

    # ==================================================================
    # phase 1c: time mix
    # ==================================================================
    TCn = g.tc
    with contextlib.ExitStack() as p1:
        def pool1(name, bufs, space="SBUF"):
            return p1.enter_context(
                tc.tile_pool(name=name, bufs=bufs, space=space))

        smw = pool1("smw", 1)
        c_mw1 = smw.tile([P, g.ct, 5 * g.tm], F16)
        dma(out=c_mw1[:], in_=tn["mw1"][:, :, :])
        c_mw2a = smw.tile([4 * g.tm, g.c], F16)
        dma(out=c_mw2a[:], in_=tn["mw2a"][:, :])
        c_mw2b = smw.tile([g.tm, g.c], F16)
        dma(out=c_mw2b[:], in_=tn["mw2b"][:, :])
        c_tdw1 = smw.tile([P, g.ct, g.td], F16)
        dma(out=c_tdw1[:], in_=tn["tdw1"][:, :, :])
        c_tdw2 = smw.tile([g.td, g.oc], F16)
        dma(out=c_tdw2[:], in_=tn["tdw2"][:, :])

        vp = pool1("vars", 2)
        t0p = pool1("t0", 3)
        rk = pool1("rkvgw", 2)
        wkvp = pool1("wkv", 2)
        wlh = pool1("wlh", 3)
        wvg = pool1("wvg", 3)
        ogp = pool1("og", 2)
        evp = pool1("evac", 3)
        S_pool = pool1("state", 2)
        gnp = pool1("gn", 2)
        pp_m = pool1("pp_m", 2, space="PSUM")
        pp_w = pool1("pp_w", 4, space="PSUM")

        S_cur = S_pool.tile([P, g.npair, g.hs], F32, tag="S")
        nc.vector.memset(S_cur, 0.0)

        def dbg_dump(name, ap, dram_ap=None, cast=True):
            if not tn["debug"]:
                return
            if cast:
                tt = gnp.tile(list(ap.shape), F32, tag="dbg")
                nc.vector.tensor_copy(out=tt[:], in_=ap)
                ap = tt[:]
            dma(out=tn[name].ap() if dram_ap is None else dram_ap, in_=ap)

        for sc in range(g.nsc):
            t0 = sc * TCn
            tsl = slice(t0, t0 + TCn)

            # ---- mix variant for xxx ----
            vx = vp.tile([P, g.ct, TCn], F16, tag="var")
            for j in range(g.ct):
                eng = nc.vector
                eng.scalar_tensor_tensor(out=vx[:, j, :], in0=xx[:, j, tsl],
                                         scalar=coef_col(0, j),
                                         in1=xn[:, j, tsl],
                                         op0=OP.mult, op1=OP.add)
            # ---- xxx = tanh(vx @ mw1): (160, TCn) as 128 + 32 ----
            xxa = t0p.tile([P, TCn], F16, tag="xxa")
            xxb = t0p.tile([g.tm, TCn], F16, tag="xxb")
            pa = pp_m.tile([P, TCn], F32, tag="m")
            for j in range(g.ct):
                nc.tensor.matmul(pa[:], c_mw1[:, j, 0:P], vx[:, j, :],
                                 start=(j == 0), stop=(j == g.ct - 1))
            nc.scalar.activation(out=xxa[:], in_=pa[:], func=AF.Tanh)
            pb = pp_m.tile([g.tm, TCn], F32, tag="m")
            for j in range(g.ct):
                nc.tensor.matmul(pb[:], c_mw1[:, j, P:5 * g.tm], vx[:, j, :],
                                 start=(j == 0), stop=(j == g.ct - 1))
            nc.scalar.activation(out=xxb[:], in_=pb[:], func=AF.Tanh)

            rT = rk.tile([P, g.oct, TCn], F16, tag="rT")
            kT = rk.tile([P, g.oct, TCn], F16, tag="kT")
            wT = rk.tile([P, g.oct, TCn], F32, tag="wT")
            v_tm = rk.tile([P, g.nl, g.oc], F32, tag="v_tm")
            g_tm = rk.tile([P, g.nl, g.oc], F16, tag="g_tm")

            def build_variant(f):
                var = vp.tile([P, g.ct, TCn], F16, tag="var")
                for j in range(g.ct):
                    mp = pp_m.tile([P, TCn], F32, tag="m")
                    if f < 4:
                        nc.tensor.matmul(mp[:],
                                         c_mw2a[bass.ts(f, g.tm),
                                                bass.ts(j, P)],
                                         xxa[bass.ts(f, g.tm), :],
                                         start=True, stop=True,
                                         tile_position=(f * g.tm, 0))
                    else:
                        nc.tensor.matmul(mp[:], c_mw2b[:, bass.ts(j, P)],
                                         xxb[:], start=True, stop=True)
                    tt = t0p.tile([P, TCn], F16, tag="t0")
                    nc.scalar.activation(out=tt[:], in_=mp[:],
                                         func=AF.Identity,
                                         bias=coef_col(1 + f, j), scale=1.0)
                    eng = nc.vector if j % 2 == 0 else nc.gpsimd
                    eng.tensor_mul(out=var[:, j, :], in0=tt[:],
                                   in1=xx[:, j, tsl])
                    eng2 = nc.gpsimd if j % 2 == 0 else nc.vector
                    eng2.tensor_add(out=var[:, j, :], in0=var[:, j, :],
                                    in1=xn[:, j, tsl])
                return var

            def proj_cm(var, wname, out_t, o):
                wt = wlh.tile([P, g.ct, P], F16, tag="wl")
                dma(out=wt[:], in_=tn[wname][o, :, :, :])
                po = pp_proj.tile([P, 512], F32, tag="proj")
                for j in range(g.ct):
                    nc.tensor.matmul(po[:, 0:TCn], wt[:, j, :], var[:, j, :],
                                     start=(j == 0), stop=(j == g.ct - 1))
                nc.scalar.copy(out=out_t[:, o, :], in_=po[:, 0:TCn])

            def proj_tm(var, wname, evac):
                """time-major projection: j outer, nl psum tiles."""
                pos = [pp_proj.tile([P, 512], F32, name=f"po{i}", tag="proj")
                       for i in range(g.nl)]
                for j in range(g.ct):
                    wt = wvg.tile([P, g.oc], F16, tag="wvg")
                    dma(out=wt[:], in_=tn[wname][:, j, :])
                    for l in range(g.nl):
                        nc.tensor.matmul(pos[l][0:g.l, 0:g.oc],
                                         var[:, j, bass.ts(l, g.l)], wt[:],
                                         start=(j == 0),
                                         stop=(j == g.ct - 1))
                for l in range(g.nl):
                    evac(l, pos[l][0:g.l, 0:g.oc])

            # --- w path ---
            var_w = build_variant(0)
            pw = pp_m.tile([g.td, TCn], F32, tag="m")
            for j in range(g.ct):
                nc.tensor.matmul(pw[:], c_tdw1[:, j, :], var_w[:, j, :],
                                 start=(j == 0), stop=(j == g.ct - 1))
            th = t0p.tile([g.td, TCn], F16, tag="th")
            nc.scalar.activation(out=th[:], in_=pw[:], func=AF.Tanh)
            for o in range(g.oct):
                po = pp_proj.tile([P, 512], F32, tag="proj")
                nc.tensor.matmul(po[:, 0:TCn], c_tdw2[:, bass.ts(o, P)], th[:],
                                 start=True, stop=True)
                nc.scalar.activation(out=wT[:, o, :], in_=po[:, 0:TCn],
                                     func=AF.Identity,
                                     bias=c_tdec[:, o:o + 1], scale=1.0)

            # --- k, v, r, g ---
            var_k = build_variant(1)
            for o in range(g.oct):
                proj_cm(var_k, "wkT", kT, o)
            var_v = build_variant(2)

            def v_evac(l, po):
                nc.scalar.copy(out=v_tm[0:g.l, l, :], in_=po)
            proj_tm(var_v, "wvT", v_evac)
            var_r = build_variant(3)
            for o in range(g.oct):
                proj_cm(var_r, "wrT", rT, o)
            var_g = build_variant(4)

            def g_evac(l, po):
                sgt = t0p.tile([P, g.oc], F32, tag="sg")
                nc.scalar.activation(out=sgt[0:g.l, :], in_=po,
                                     func=AF.Sigmoid)
                nc.vector.tensor_mul(out=g_tm[0:g.l, l, :],
                                     in0=sgt[0:g.l, :], in1=po)
            proj_tm(var_g, "wgT", g_evac)

            if sc == 0 and tn["debug"]:
                dbg_dump("dbg_vark", var_k[:, :, :],
                         tn["dbg_vark"].ap().rearrange("(j p) t -> p j t",
                                                       p=P))
                dbg_dump("dbg_r", rT[:, :, :],
                         tn["dbg_r"].ap().rearrange("(o p) t -> p o t", p=P))
                dbg_dump("dbg_k", kT[:, :, :],
                         tn["dbg_k"].ap().rearrange("(o p) t -> p o t", p=P))
                dbg_dump("dbg_w", wT[:, :, :],
                         tn["dbg_w"].ap().rearrange("(o p) t -> p o t", p=P),
                         cast=False)
                for l in range(g.nl):
                    dbg_dump("dbg_v", v_tm[0:g.l, l, :],
                             tn["dbg_v"][bass.ts(l, g.l), :], cast=False)
                    dbg_dump("dbg_g", g_tm[0:g.l, l, :],
                             tn["dbg_g"][bass.ts(l, g.l), :])
                dbg_dump("dbg_xn", xn[:, :, :],
                         tn["dbg_xn"].ap().rearrange("(j p) t -> p j t", p=P))
                dbg_dump("dbg_xx", xx[:, :, :],
                         tn["dbg_xx"].ap().rearrange("(j p) t -> p j t", p=P))

            # ---- WKV chunks + GroupNorm + og ----
            ogT = ogp.tile([P, g.oct, TCn], F16, tag="ogT")
            if tn["phases"] == 21:
                nc.vector.memset(ogT, 0.0)
            WLVL = tn["phases"] - 220 if 220 <= tn["phases"] <= 223 else 9
            WSUB = tn["phases"] - 2210 if 2210 <= tn["phases"] <= 2213 else 9
            if tn["phases"] in (22101, 22102): WLVL = 9
            if WSUB != 9:
                WLVL = 1
            for l in range(g.nl if tn["phases"] != 21 else 0):
                lsl = slice(l * g.l, (l + 1) * g.l)
                O_sb = gnp.tile([g.l, g.oct, P], F32, tag="O")
                S_nxt = S_pool.tile([P, g.npair, g.hs], F32, tag="S")
                for p in range(g.npair):
                    ex = wkvp.tile([P, g.l], F32, tag="ex")
                    nc.scalar.activation(out=ex[:], in_=wT[:, p, lsl],
                                         func=AF.Exp)
                    lam = wkvp.tile([P, g.l], F32, tag="lam")
                    nc.vector.tensor_tensor_scan(out=lam[:], data0=ex[:],
                                                 data1=c_neg1[:, 0:g.l],
                                                 initial=0.0,
                                                 op0=OP.subtract, op1=OP.mult)
                    e2 = wkvp.tile([P, g.l], F32, tag="e2")
                    nc.scalar.activation(out=e2[:], in_=lam[:], func=AF.Exp,
                                         scale=-1.0)
                    e1s = wkvp.tile([P, g.l], F32, tag="e1s")
                    nc.vector.memset(e1s[:, 0:1], 1.0)
                    nc.scalar.activation(out=e1s[:, 1:g.l],
                                         in_=lam[:, 0:g.l - 1], func=AF.Exp)
                    pl = wkvp.tile([P, 1], F32, tag="pl")
                    nc.scalar.activation(out=pl[:], in_=lam[:, g.l - 1:g.l],
                                         func=AF.Exp)
                    rt = wkvp.tile([P, g.l], F32, tag="rt")
                    nc.gpsimd.tensor_mul(out=rt[:], in0=rT[:, p, lsl],
                                         in1=e1s[:])
                    kt = wkvp.tile([P, g.l], F32, tag="kt")
                    nc.vector.tensor_mul(out=kt[:], in0=kT[:, p, lsl],
                                         in1=e2[:])
                    khat = wkvp.tile([P, g.l], F32, tag="khat")
                    nc.vector.tensor_scalar_mul(out=khat[:], in0=kt[:],
                                                scalar1=pl[:])
                    rukp = wkvp.tile([P, g.l], F32, tag="rukp")
                    nc.vector.scalar_tensor_tensor(out=rukp[:],
                                                   in0=rT[:, p, lsl],
                                                   scalar=c_u[:, p:p + 1],
                                                   in1=kT[:, p, lsl],
                                                   op0=OP.mult, op1=OP.mult)
                    if WLVL == 0:
                        nc.vector.memset(O_sb[:, p, :], 0.0)
                        nc.vector.memset(S_nxt[:, p, :], 0.0)
                        continue
                    ktm = wkvp.tile([g.l, P], F32, tag="ktm")
                    pkt = pp_w.tile([g.l, P], F32, tag="wkv")
                    nc.tensor.transpose(pkt[:], khat[:, :], c_id32[:, :])
                    nc.vector.tensor_copy(out=ktm[:], in_=pkt[:])
                    if sc == 0 and l == 0 and p == 0 and tn["debug"]:
                        dbg_dump("dbg_ktm", ktm[:, :],
                                 tn["dbg_ktm"][0:g.l, :], cast=False)
                        dbg_dump("dbg_lam", lam[:, :],
                                 tn["dbg_lam"][:, 0:g.l], cast=False)
                    if WSUB == 0:
                        nc.vector.memset(O_sb[:, p, :], 0.0)
                        nc.vector.memset(S_nxt[:, p, :], 0.0)
                        continue
                    pA_h = [pp_w.tile([g.l, g.l], F32, name=f"pA{i}",
                                      tag="wkv") for i in range(2)]
                    for hh in range(2):
                        hsl = slice(hh * g.hs, (hh + 1) * g.hs)
                        nc.tensor.matmul(pA_h[hh][:, :],
                                         kt[hsl, :], rt[hsl, :],
                                         start=True, stop=True)
                    if WSUB == 1:
                        am_ = wkvp.tile([g.l, 2 * g.l], F32, tag="Am")
                        for hh in range(2):
                            nc.vector.tensor_mul(out=am_[:, bass.ts(hh, g.l)],
                                                 in0=pA_h[hh][:, :],
                                                 in1=c_mask[:, :])
                        nc.vector.memset(O_sb[:, p, :], 0.0)
                        nc.vector.memset(S_nxt[:, p, :], 0.0)
                        continue
                    pruk_h = [pp_w.tile([g.l, 1], F32, name=f"pruk{i}",
                                        tag="wkv") for i in range(2)]
                    for hh in range(2):
                        hsl = slice(hh * g.hs, (hh + 1) * g.hs)
                        nc.tensor.matmul(pruk_h[hh][:, :], rukp[hsl, :],
                                         c_ones_f[hsl, :],
                                         start=True, stop=True)
                    if WSUB == 2:
                        nc.vector.memset(O_sb[:, p, :], 0.0)
                        nc.vector.memset(S_nxt[:, p, :], 0.0)
                        continue
                    ruk_sb = wkvp.tile([g.l, 2], F32, tag="ruks")
                    nc.vector.tensor_copy(out=ruk_sb[:, 0:1], in_=pruk_h[0][:])
                    nc.vector.tensor_copy(out=ruk_sb[:, 1:2], in_=pruk_h[1][:])
                    Am = wkvp.tile([g.l, 2 * g.l], F32, tag="Am")
                    for hh in range(2):
                        nc.vector.tensor_mul(out=Am[:, bass.ts(hh, g.l)],
                                             in0=pA_h[hh][:, :],
                                             in1=c_mask[:, :])
                    if WLVL == 1:
                        nc.vector.memset(O_sb[:, p, :], 0.0)
                        nc.vector.memset(S_nxt[:, p, :], 0.0)
                        continue
                    pO_h = [pp_w.tile([g.l, g.hs], F32, name=f"pO{i}",
                                      tag="wkv") for i in range(2)]
                    for hh in range(2):
                        hsl = slice(hh * g.hs, (hh + 1) * g.hs)
                        csl = slice((2 * p + hh) * g.hs,
                                    (2 * p + hh + 1) * g.hs)
                        nc.tensor.matmul(pO_h[hh][:, :],
                                         Am[:, bass.ts(hh, g.l)],
                                         v_tm[0:g.l, l, csl],
                                         start=True, stop=False)
                        nc.tensor.matmul(pO_h[hh][:, :],
                                         rt[hsl, :], S_cur[hsl, p, :],
                                         start=False, stop=True)
                    for hh in range(2):
                        hsl = slice(hh * g.hs, (hh + 1) * g.hs)
                        csl = slice((2 * p + hh) * g.hs,
                                    (2 * p + hh + 1) * g.hs)
                        nc.vector.scalar_tensor_tensor(
                            out=O_sb[:, p, hsl],
                            in0=v_tm[0:g.l, l, csl],
                            scalar=ruk_sb[:, hh:hh + 1],
                            in1=pO_h[hh][:, :],
                            op0=OP.mult, op1=OP.add)
                    if WLVL == 2:
                        nc.vector.memset(S_nxt[:, p, :], 0.0)
                        continue
                    pS = pp_w.tile([P, g.hs], F32, tag="wkv")
                    for hh in range(2):
                        hsl = slice(hh * g.hs, (hh + 1) * g.hs)
                        csl = slice((2 * p + hh) * g.hs,
                                    (2 * p + hh + 1) * g.hs)
                        nc.tensor.matmul(pS[hsl, :], ktm[:, hsl],
                                         v_tm[0:g.l, l, csl],
                                         start=True, stop=True,
                                         skip_group_check=True)
                    tS = wkvp.tile([P, g.hs], F32, tag="tS")
                    nc.vector.tensor_scalar_mul(out=tS[:],
                                                in0=S_cur[:, p, :],
                                                scalar1=pl[:])
                    nc.vector.tensor_add(out=S_nxt[:, p, :], in0=tS[:],
                                         in1=pS[:])
                if sc == 0 and l == 0 and tn["debug"]:
                    dbg_dump("dbg_S",
                             S_nxt[:, :, :].rearrange("p a b -> p (a b)"),
                             tn["dbg_S"][:, 0:g.npair * g.hs], cast=False)
                S_cur = S_nxt

                if tn["phases"] == 22:
                    nc.vector.memset(ogT[:, :, lsl], 0.0)
                    continue
                # GroupNorm over each head (free dim), then *g, transpose
                Ov = O_sb[:, :, :].rearrange("t a b -> t (a b)").rearrange(
                    "t (h d) -> t h d", d=g.hs)
                Of = O_sb[:, :, :].rearrange("t a b -> t (a b)")
                if tn["debug"]:
                    dbg_dump("dbg_o", Of,
                             tn["dbg_o"][sc * TCn + l * g.l:
                                         sc * TCn + (l + 1) * g.l, :],
                             cast=False)
                sums = gnp.tile([g.l, g.nh], F32, tag="sums")
                nc.vector.reduce_sum(out=sums[:], in_=Ov, axis=AX.X)
                osq = gnp.tile([g.l, g.oc], F32, tag="osq")
                nc.scalar.square(out=osq[:], in_=Of)
                sqs = gnp.tile([g.l, g.nh], F32, tag="sqs")
                nc.vector.reduce_sum(out=sqs[:],
                                     in_=osq.rearrange("t (h d) -> t h d",
                                                       d=g.hs), axis=AX.X)
                mean = gnp.tile([g.l, g.nh], F32, tag="mean")
                nc.scalar.mul(out=mean[:], in_=sums[:], mul=1.0 / g.hs)
                var_ = gnp.tile([g.l, g.nh], F32, tag="var")
                nc.scalar.mul(out=var_[:], in_=sqs[:], mul=1.0 / g.hs)
                msq = gnp.tile([g.l, g.nh], F32, tag="msq")
                nc.vector.tensor_mul(out=msq[:], in0=mean[:], in1=mean[:])
                nc.vector.tensor_sub(out=var_[:], in0=var_[:], in1=msq[:])
                nc.scalar.activation(out=var_[:], in_=var_[:], func=AF.Sqrt,
                                     bias=c_eps[0:g.l, :], scale=1.0)
                rstd = gnp.tile([g.l, g.nh], F32, tag="rstd")
                nc.vector.reciprocal(out=rstd[:], in_=var_[:])
                og16 = gnp.tile([g.l, g.oc], F16, tag="og16")
                ogv = og16.rearrange("t (h d) -> t h d", d=g.hs)
                for h in range(g.nh):
                    eng = nc.vector
                    eng.tensor_scalar(out=ogv[:, h, :], in0=Ov[:, h, :],
                                      scalar1=mean[:, h:h + 1],
                                      scalar2=rstd[:, h:h + 1],
                                      op0=OP.subtract, op1=OP.mult)
                nc.vector.tensor_mul(out=og16[:], in0=og16[:],
                                     in1=g_tm[0:g.l, l, :])
                if sc == 0 and l == 0 and tn["debug"]:
                    dbg_dump("dbg_og", og16[:, :])
                if tn["phases"] == 23:
                    nc.vector.memset(ogT[:, :, lsl], 0.0)
                    continue
                for p in range(g.oct):
                    pt = pp_w.tile([P, g.l], F16, tag="wkv")
                    nc.tensor.transpose(pt[:], og16[:, bass.ts(p, P)],
                                        c_id[0:g.l, 0:g.l])
                    nc.vector.tensor_copy(out=ogT[:, p, lsl], in_=pt[:])

            if sc == 0 and tn["debug"]:
                dbg_dump("dbg_ogT", ogT[:, :, :],
                         tn["dbg_ogT"].ap().rearrange("(o p) t -> p o t", p=P))

            # ---- Wo partials ----
            for ot in range(g.ct):
                wt = wlh.tile([P, g.oct, P], F16, tag="wo")
                dma(out=wt[:], in_=tn["woT"][ot, :, :, :])
                po = pp_proj.tile([P, 512], F32, tag="proj")
                for p in range(g.oct):
                    nc.tensor.matmul(po[:, 0:TCn], wt[:, p, :], ogT[:, p, :],
                                     start=(p == 0), stop=(p == g.oct - 1))
                ev = evp.tile([P, TCn], F32, tag="ev")
                nc.scalar.copy(out=ev[:], in_=po[:, 0:TCn])
                dma(out=tn["cc1_in"][bass.ts(ot, P), tsl], in_=ev[:])

    if tn["phases"] in (2, 21, 22, 23) or 220 <= tn["phases"] <= 223 or 2210 <= tn["phases"] <= 2213 or tn["phases"] in (22101, 22102):
        fin = ctx.enter_context(tc.tile_pool(name="fin2", bufs=2))
        for o in range(g.oct):
            tt_ = fin.tile([P, g.t], F32, tag="f")
            dma(out=tt_[:], in_=tn["cc1_in"][bass.ts(o, P), :])
            dma(out=tn["yT"][bass.ts(o, P), :], in_=tt_[:])
        return

    # ==================================================================
    # collectives on attention partials
    # ==================================================================
    nc.gpsimd.collective_compute(
        "AllReduce", OP.add, replica_groups=tn["groups"],
        ins=[tn["cc1_in"].ap().opt()], outs=[tn["cc1_out"].ap().opt()])
    nc.gpsimd.collective_compute(
        "ReduceScatter", OP.add, replica_groups=tn["groups"],
        ins=[tn["cc1_in"].ap().opt()], outs=[tn["cc1b_out"].ap().opt()])

    # ==================================================================
    # phase 2a: LN2 -> xn2, xx2  (xmid streamed: cc1_out + xT)
    # ==================================================================
    with tc.tile_pool(name="sq2", bufs=2) as sq2, \
         tc.tile_pool(name="row2", bufs=3) as row2, \
         tc.tile_pool(name="pps2", bufs=2, space="PSUM") as pps2:

        def m_chunk(sqp, j, csl):
            ta = sqp.tile([P, NT5], F32, tag="ldc")
            dma(out=ta[:], in_=tn["cc1_out"][bass.ts(j, P), csl])
            tb = sqp.tile([P, NT5], F32, tag="ldc2")
            dma(out=tb[:], in_=tn["xT"][bass.ts(j, P), csl])
            tm_ = sqp.tile([P, NT5], F32R, tag="ldm")
            eng = nc.vector if j % 2 == 0 else nc.gpsimd
            eng.tensor_add(out=tm_[:], in0=ta[:], in1=tb[:])
            return tm_[:]

        def m_row(sqp, j):
            ta = sqp.tile([P, g.t], F32, tag="ldr")
            dma(out=ta[:], in_=tn["cc1_out"][bass.ts(j, P), :])
            tb = sqp.tile([P, g.t], F32, tag="ldr2")
            dma(out=tb[:], in_=tn["xT"][bass.ts(j, P), :])
            eng = nc.vector if j % 2 == 0 else nc.gpsimd
            eng.tensor_add(out=ta[:], in0=ta[:], in1=tb[:])
            return ta[:]

        ln_stats(m_chunk, sq2, row2, pps2, 2, g.c)
        bcast_stats(2)
        ln_apply(m_row, sq2, xn, xx, 9)

    xn2, xx2 = xn, xx
    if tn["phases"] == 3:
        fin = ctx.enter_context(tc.tile_pool(name="fin3", bufs=2))
        for o in range(g.oct):
            tt_ = fin.tile([P, g.t], F32, tag="f")
            nc.vector.tensor_copy(out=tt_[:], in_=xn2[:, o, :])
            dma(out=tn["yT"][bass.ts(o, P), :], in_=tt_[:])
        return

    # ==================================================================
    # phase 2b: FFN
    # ==================================================================
    TC2 = g.tc2
    sigp = ctx.enter_context(tc.tile_pool(name="sig", bufs=1))
    sig_t = sigp.tile([P, g.oct, g.t], F16)
    with contextlib.ExitStack() as p2:
        def pool2(name, bufs, space="SBUF"):
            return p2.enter_context(
                tc.tile_pool(name=name, bufs=bufs, space=space))

        kfp = pool2("kf", 2)
        v2p = pool2("v2", 1)
        wlh2 = pool2("wlh2", 2)
        ev2p = pool2("ev2", 3)
        t2p = pool2("t2", 2)

        for sc in range(g.nsc2):
            t0 = sc * TC2
            tsl = slice(t0, t0 + TC2)
            vk2 = v2p.tile([P, g.ct, TC2], F16, tag="v2")
            for j in range(g.ct):
                eng = nc.vector
                eng.scalar_tensor_tensor(out=vk2[:, j, :], in0=xx2[:, j, tsl],
                                         scalar=coef_col(6, j),
                                         in1=xn2[:, j, tsl],
                                         op0=OP.mult, op1=OP.add)
            kf = kfp.tile([P, g.fct, TC2], F16, tag="kf")
            for ft in range(g.fct):
                wt = wlh2.tile([P, g.ct, P], F16, tag="wl2")
                dma(out=wt[:], in_=tn["wkfT"][ft, :, :, :])
                po = pp_proj.tile([P, 512], F32, tag="proj")
                for j in range(g.ct):
                    nc.tensor.matmul(po[:, 0:TC2], wt[:, j, :], vk2[:, j, :],
                                     start=(j == 0), stop=(j == g.ct - 1))
                tr = t2p.tile([P, TC2], F16, tag="relu")
                nc.scalar.activation(out=tr[:], in_=po[:, 0:TC2], func=AF.Relu)
                eng = nc.vector if ft % 2 == 0 else nc.gpsimd
                eng.tensor_mul(out=kf[:, ft, :], in0=tr[:], in1=tr[:])
            for ot in range(g.ct):
                wt = wlh2.tile([P, g.fct, P], F16, tag="wvf")
                dma(out=wt[:], in_=tn["wvfT"][ot, :, :, :])
                po = pp_proj.tile([P, 512], F32, tag="proj")
                for ft in range(g.fct):
                    nc.tensor.matmul(po[:, 0:TC2], wt[:, ft, :], kf[:, ft, :],
                                     start=(ft == 0), stop=(ft == g.fct - 1))
                ev = ev2p.tile([P, TC2], F32, tag="ev2")
                nc.scalar.copy(out=ev[:], in_=po[:, 0:TC2])
                dma(out=tn["cc2_in"][bass.ts(ot, P), tsl], in_=ev[:])
            vr2 = v2p.tile([P, g.ct, TC2], F16, tag="v2")
            for j in range(g.ct):
                eng = nc.vector
                eng.scalar_tensor_tensor(out=vr2[:, j, :], in0=xx2[:, j, tsl],
                                         scalar=coef_col(7, j),
                                         in1=xn2[:, j, tsl],
                                         op0=OP.mult, op1=OP.add)
            for o in range(g.oct):
                wt = wlh2.tile([P, g.ct, P], F16, tag="wl2")
                dma(out=wt[:], in_=tn["wrfT"][o, :, :, :])
                po = pp_proj.tile([P, 512], F32, tag="proj")
                for j in range(g.ct):
                    nc.tensor.matmul(po[:, 0:TC2], wt[:, j, :], vr2[:, j, :],
                                     start=(j == 0), stop=(j == g.ct - 1))
                nc.scalar.activation(out=sig_t[:, o, tsl], in_=po[:, 0:TC2],
                                     func=AF.Sigmoid)

        nc.gpsimd.collective_compute(
            "ReduceScatter", OP.add, replica_groups=tn["groups"],
            ins=[tn["cc2_in"].ap().opt()], outs=[tn["cc2_out"].ap().opt()])

        # final: y = (x_sl + att_sl) + sig * kv
        for o in range(g.oct):
            kv = ev2p.tile([P, g.t], F32, tag="kvl")
            dma(out=kv[:], in_=tn["cc2_out"][bass.ts(o, P), :])
            xs = ev2p.tile([P, g.t], F32, tag="xsl")
            dma(out=xs[:], in_=tn["x_sl"][bass.ts(o, P), :])
            at = ev2p.tile([P, g.t], F32, tag="atl")
            dma(out=at[:], in_=tn["cc1b_out"][bass.ts(o, P), :])
            nc.vector.tensor_mul(out=kv[:], in0=sig_t[:, o, :], in1=kv[:])
            nc.vector.tensor_add(out=xs[:], in0=xs[:], in1=at[:])
            nc.vector.tensor_add(out=kv[:], in0=kv[:], in1=xs[:])
            dma(out=tn["yT"][bass.ts(o, P), :], in_=kv[:])


# ---------------------------------------------------------------------------
# host-side sharding / gather
# ---------------------------------------------------------------------------

def shard_inputs(g: CFG, inputs):
    """inputs: full setup_inputs() dict (numpy).  Returns list of per-core
    input maps."""
    f16 = np.float16
    f32 = np.float32

    def a(x):
        return np.ascontiguousarray(x)

    x = inputs["x"].astype(f32)
    assert not np.any(inputs["ln1_b"]) and not np.any(inputs["ln2_b"]), \
        "kernel assumes zero LN bias"
    assert not np.any(inputs["lnx_b"]), "kernel assumes zero lnx_b"

    def lhsT_bank(wT, oct_, ct_):
        # wT: (c_in, c_out_local) -> (oct, 128kin, ct, 128m)
        cin, cout = wT.shape
        r = wT.reshape(ct_, P, oct_, P)        # (j, kin, o, m)
        return a(r.transpose(2, 1, 0, 3).astype(f16))

    maps = []
    for core in range(g.n_cores):
        b = core // g.tpg
        r = core % g.tpg
        osl = slice(r * g.oc, (r + 1) * g.oc)
        fsl = slice(r * g.fc, (r + 1) * g.fc)
        hsl = slice(r * g.nh, (r + 1) * g.nh)
        m = {}
        m["xT"] = a(x[b].T)
        m["x_sl"] = a(x[b].T[osl])
        m["wrT"] = lhsT_bank(inputs["Wr"].T[:, osl].astype(f32), g.oct, g.ct)
        m["wkT"] = lhsT_bank(inputs["Wk"].T[:, osl].astype(f32), g.oct, g.ct)
        # woT: rows = local og channels, cols = full C; fold lnx_w
        woT = (inputs["lnx_w"][osl, None] * inputs["Wo"].T[osl, :]).astype(f32)
        # bank layout (ct outer, 128kin(local og), oct inner, m): transpose of
        # lhsT_bank with swapped roles: out-cols are full C (ct tiles)
        rr = woT.reshape(g.oct, P, g.ct, P)    # (p, kin, ot, m)
        m["woT"] = a(rr.transpose(2, 1, 0, 3).astype(f16))
        m["wkfT"] = lhsT_bank(inputs["Wk_ffn"].T[:, fsl].astype(f32),
                              g.fct, g.ct)
        rr = inputs["Wv_ffn"].T[fsl, :].astype(f32).reshape(g.fct, P, g.ct, P)
        m["wvfT"] = a(rr.transpose(2, 1, 0, 3).astype(f16))
        m["wrfT"] = lhsT_bank(inputs["Wr_ffn"].T[:, osl].astype(f32),
                              g.oct, g.ct)
        m["wvT"] = a(inputs["Wv"].T[:, osl].astype(f32).reshape(
            g.ct, P, g.oc).transpose(1, 0, 2).astype(f16))
        m["wgT"] = a(inputs["Wg"].T[:, osl].astype(f32).reshape(
            g.ct, P, g.oc).transpose(1, 0, 2).astype(f16))
        m["tdw1"] = a(inputs["time_decay_w1"].astype(f32).reshape(
            g.ct, P, g.td).transpose(1, 0, 2).astype(f16))
        m["tdw2"] = a(inputs["time_decay_w2"][:, osl].astype(f16))
        m["mw1"] = a(inputs["time_maa_w1"].astype(f32).reshape(
            g.ct, P, 5 * g.tm).transpose(1, 0, 2).astype(f16))
        mw2 = inputs["time_maa_w2"].astype(f32)   # (5, tm, c)
        m["mw2a"] = a(mw2[:4].reshape(4 * g.tm, g.c).astype(f16))
        m["mw2b"] = a(mw2[4].astype(f16))
        coef = np.zeros((P, 10, g.ct), f32)
        for i, nm in enumerate(["time_maa_x", "time_maa_w", "time_maa_k",
                                "time_maa_v", "time_maa_r", "time_maa_g",
                                "ffn_maa_k", "ffn_maa_r", "ln1_w", "ln2_w"]):
            coef[:, i, :] = inputs[nm].astype(f32).reshape(g.ct, P).T
        m["coef"] = a(coef)
        m["tdec"] = a(inputs["time_decay"].astype(f32)[osl].reshape(
            g.oct, P).T)
        u = inputs["time_faaaa"].astype(f32).reshape(-1)[
            r * g.oc:(r + 1) * g.oc]
        m["u"] = a(u.reshape(g.npair, P).T)
        m["mask_su"] = a(np.triu(np.ones((g.l, g.l), f32), 1))
        m["ident"] = a(np.eye(P, dtype=f16))
        m["ones_r"] = a(np.ones((P, 1), f32))
        maps.append(m)
    return maps


def assemble(g: CFG, results):
    out = np.empty((g.b, g.t, g.c), np.float32)
    for core, res in enumerate(results):
        b = core // g.tpg
        r = core % g.tpg
        out[b, :, r * g.oc:(r + 1) * g.oc] = res["yT"].T
    return out


# ---------------------------------------------------------------------------
# public entry point
# ---------------------------------------------------------------------------

@functools.lru_cache(maxsize=1)
def _get_nc():
    nc = build_nc(FULL)
    nc.compile()
    return nc


def kernel(**inputs):
    from concourse.bass_utils import run_bass_kernel_spmd
    g = FULL
    nc = _get_nc()
    in_maps = shard_inputs(g, {k: np.asarray(v) for k, v in inputs.items()})
    res = run_bass_kernel_spmd(nc, in_maps, core_ids=list(range(g.n_cores)))
    return assemble(g, res.results)



# revision 19
# speedup vs baseline: 1.6224x; 1.6224x over previous
"""RWKV6 (x060) block kernel for Trainium2 across 8 NeuronCores.

Sharding: DP2 x TP4.  Cores 0-3 compute batch 0, cores 4-7 batch 1.
Within each group of 4 cores: heads (8 per core), projection output
channels (C/4=512) and the FFN dim (FFN/4=1792) are tensor-parallel.

v1 collective plan (replaces the old AllReduce):
  per time-chunk sc (512): ReduceScatter of fp16 Wo partials -> local
  residual slice; xmid16 = x_sl + att_sl; AllGather of fp16 xmid ->
  full (C, 512) per half for LN2 + FFN contraction; fp16 ReduceScatter
  of FFN value partials.  All chunked so they overlap compute.

Engine notes: silu/sigmoid are computed via tanh (activation set 0 =
{exp, tanh, identity, relu, square}) so the scalar engine never swaps
activation tables; LN/GN rstd uses the DVE pow ALU op ((var+eps)^-0.5).
The 0.5 factors from the tanh forms of silu/sigmoid are folded into the
Wo / Wv_ffn weights host-side.
"""

import contextlib
import functools
import numpy as np

import concourse.bacc as bacc
import concourse.bass as bass
import concourse.mybir as mybir
import concourse.tile as tile

F32 = mybir.dt.float32
F32R = mybir.dt.float32r
F16 = mybir.dt.float16
AX = mybir.AxisListType
OP = mybir.AluOpType
AF = mybir.ActivationFunctionType

P = 128
EPS_LN = 1e-5
EPS_GN = 64e-5


class CFG:
    def __init__(self, b=2, t=1024, c=2048, hs=64, h=32, ffn=7168, tm=32, td=64,
                 tpg=4, l=128, tc=512):
        self.b, self.t, self.c, self.hs, self.h = b, t, c, hs, h
        self.ffn, self.tm, self.td = ffn, tm, td
        self.tpg = tpg
        self.l = l
        self.tc = tc
        self.n_cores = 2 * tpg
        self.ct = c // P
        self.nh = h // tpg
        self.npair = self.nh // 2
        self.oc = c // tpg
        self.oct = self.oc // P
        self.fc = ffn // tpg
        self.fct = self.fc // P
        self.nsc = t // tc
        self.nl = tc // l
        assert self.nh % 2 == 0 and hs == 64 and self.oc % P == 0
        assert self.fc % P == 0 and t % tc == 0 and tc % l == 0


FULL = CFG()


# ---------------------------------------------------------------------------
# builder
# ---------------------------------------------------------------------------

def build_nc(g: CFG, debug=False, phases=9):
    nc = bacc.Bacc("TRN2", target_bir_lowering=False, num_devices=g.n_cores)

    def din(name, shape, dt=F16):
        return nc.dram_tensor(name, list(shape), dt, kind="ExternalInput")

    tn = {}
    tn["xT"] = din("xT", (g.c, g.t), F32)
    tn["x_sl"] = din("x_sl", (g.oc, g.t), F32)
    # lhsT weight banks: layout (outer, 128 kin, inner, 128 m) contiguous
    tn["wrT"] = din("wrT", (g.oct, P, g.ct, P))
    tn["wkT"] = din("wkT", (g.oct, P, g.ct, P))
    tn["woT"] = din("woT", (g.ct, P, g.oct, P))
    tn["wkfT"] = din("wkfT", (g.fct, P, g.ct, P))
    tn["wvfT"] = din("wvfT", (g.ct, P, g.fct, P))
    tn["wrfT"] = din("wrfT", (g.oct, P, g.ct, P))
    # rhs weight banks (kin-major): (128 kin, ct, oc)
    tn["wvT"] = din("wvT", (P, g.ct, g.oc))
    tn["wgT"] = din("wgT", (P, g.ct, g.oc))
    tn["tdw1"] = din("tdw1", (P, g.ct, g.td))
    tn["tdw2"] = din("tdw2", (g.td, g.oc))
    tn["mw1"] = din("mw1", (P, g.ct, 5 * g.tm))
    tn["mw2a"] = din("mw2a", (4 * g.tm, g.c))
    tn["mw2b"] = din("mw2b", (g.tm, g.c))
    # coef rows: 0 maa_x, 1..5 maa w/k/v/r/g, 6 ffn_k, 7 ffn_r, 8 ln1_w, 9 ln2_w
    tn["coef"] = din("coef", (P, 10, g.ct), F32)
    tn["tdec"] = din("tdec", (P, g.oct), F32)
    tn["u"] = din("u", (P, g.npair), F32)
    tn["mask_su"] = din("mask_su", (g.l, g.l), F32)
    tn["ident"] = din("ident", (P, P), F16)
    tn["ones_r"] = din("ones_r", (P, 1), F32)
    tn["ones2"] = din("ones2", (P, 2), F16)

    tn["cc1_in"] = nc.dram_tensor("cc1_in", [g.nsc, g.c, g.tc], F16)
    tn["cc1_out"] = nc.dram_tensor("cc1_out", [g.nsc, g.oc, g.tc], F16)
    tn["xmid_d"] = nc.dram_tensor("xmid_d", [g.nsc, g.oc, g.tc], F16)
    tn["xag_d"] = nc.dram_tensor("xag_d", [g.nsc, g.c, g.tc], F16)
    tn["cc2_in"] = nc.dram_tensor("cc2_in", [g.nsc, g.c, g.tc], F16)
    tn["cc2_out"] = nc.dram_tensor("cc2_out", [g.nsc, g.oc, g.tc], F16)
    tn["stat_d"] = nc.dram_tensor("stat_d", [8, g.t], F32)
    tn["yT"] = nc.dram_tensor("yT", [g.oc, g.t], F32, kind="ExternalOutput")
    tn["groups"] = [list(range(g.tpg)), list(range(g.tpg, 2 * g.tpg))]

    if debug:
        for nm, shape in [("dbg_xn", (g.c, g.t)), ("dbg_r", (g.oc, g.t)),
                          ("dbg_k", (g.oc, g.t)), ("dbg_w", (g.oc, g.t)),
                          ("dbg_v", (g.t, g.oc)), ("dbg_g", (g.t, g.oc)),
                          ("dbg_o", (g.t, g.oc)), ("dbg_og", (g.t, g.oc)),
                          ("dbg_ogT", (g.oc, g.t)),
                          ("dbg_xn2", (g.c, g.t)), ("dbg_kf", (g.fc, g.t))]:
            tn[nm] = nc.dram_tensor(nm, list(shape), F32)
    tn["debug"] = debug
    tn["phases"] = phases
    with tile.TileContext(nc) as tc:
        with contextlib.ExitStack() as ctx:
            _body(ctx, nc, tc, g, tn)
    return nc


def _body(ctx, nc, tc, g, tn):
    dma = nc.sync.dma_start
    TCn = g.tc

    def pool(name, bufs, space="SBUF"):
        return ctx.enter_context(tc.tile_pool(name=name, bufs=bufs, space=space))

    # ---------------- resident constants ----------------
    cp = pool("consts", 1)
    c_coef = cp.tile([P, 10, g.ct], F32)
    dma(out=c_coef[:], in_=tn["coef"][:, :, :])
    c_tdec = cp.tile([P, g.oct], F32)
    dma(out=c_tdec[:], in_=tn["tdec"][:, :])
    c_u = cp.tile([P, g.npair], F32)
    dma(out=c_u[:], in_=tn["u"][:, :])
    c_mask = cp.tile([g.l, g.l], F32)
    dma(out=c_mask[:], in_=tn["mask_su"][:, :])
    c_id = cp.tile([P, P], F16)
    dma(out=c_id[:], in_=tn["ident"][:, :])
    c_ones_r = cp.tile([P, 1], F32R)
    dma(out=c_ones_r[:], in_=tn["ones_r"][:, :].bitcast(F32R))
    c_ones16 = cp.tile([P, 1], F16)
    nc.vector.memset(c_ones16, 1.0)
    c_ones2 = cp.tile([P, 2], F16)
    dma(out=c_ones2[:], in_=tn["ones2"][:, :])
    # 1 - coef for ffn_maa_k / ffn_maa_r (the xx2-free FFN variant trick)
    c_c1m = cp.tile([P, 2, g.ct], F32)
    nc.vector.tensor_scalar(out=c_c1m[:, 0, :], in0=c_coef[:, 6, :],
                            scalar1=-1.0, scalar2=1.0,
                            op0=OP.mult, op1=OP.add)
    nc.vector.tensor_scalar(out=c_c1m[:, 1, :], in0=c_coef[:, 7, :],
                            scalar1=-1.0, scalar2=1.0,
                            op0=OP.mult, op1=OP.add)
    c_mw1 = cp.tile([P, g.ct, 5 * g.tm], F16)
    dma(out=c_mw1[:], in_=tn["mw1"][:, :, :])
    c_mw2a = cp.tile([4 * g.tm, g.c], F16)
    dma(out=c_mw2a[:], in_=tn["mw2a"][:, :])
    c_mw2b = cp.tile([g.tm, g.c], F16)
    dma(out=c_mw2b[:], in_=tn["mw2b"][:, :])
    c_tdw1 = cp.tile([P, g.ct, g.td], F16)
    dma(out=c_tdw1[:], in_=tn["tdw1"][:, :, :])
    c_tdw2 = cp.tile([g.td, g.oc], F16)
    dma(out=c_tdw2[:], in_=tn["tdw2"][:, :])
    c_neg1 = cp.tile([P, g.l], F32)
    nc.vector.memset(c_neg1, -1.0)
    c_eps1 = cp.tile([1, 1], F32)
    nc.vector.memset(c_eps1, EPS_LN)
    c_epsg = cp.tile([P, 1], F32)
    nc.vector.memset(c_epsg, EPS_GN)

    def coef_col(idx, j):
        return c_coef[:, idx, j:j + 1]

    def dbg_dump(pl_, name, ap, dram_ap, cast=True):
        if not tn["debug"]:
            return
        if cast:
            tt = pl_.tile(list(ap.shape), F32, tag="dbg", name="dbgt")
            nc.vector.tensor_copy(out=tt[:], in_=ap)
            ap = tt[:]
        dma(out=dram_ap, in_=ap)

    # stat rows pool (b_mu/b_rho broadcast tiles)
    sbp = pool("statb", 1)

    def ln_stats_rows(psp, sqp, rowp, get_chunk, nj, tsl, nrm, eps, stat_row):
        """Streamed LN stats: mean + rstd rows for time-slice tsl.
        get_chunk(j) -> (P, TCn) AP (f32r or f16) for channel tile j."""
        ps0 = psp.tile([1, TCn], F32, tag="m", name="ps0")
        ps1 = psp.tile([1, TCn], F32, tag="m", name="ps1")
        for j in range(nj):
            xt, xsq = get_chunk(j)
            nc.tensor.matmul(ps0[:], c_ones_r, xt,
                             start=(j == 0), stop=(j == nj - 1))
            nc.tensor.matmul(ps1[:], c_ones_r, xsq,
                             start=(j == 0), stop=(j == nj - 1))
        mu = rowp.tile([1, TCn], F32, tag="rows", name="mu")
        nc.scalar.mul(out=mu[:], in_=ps0[:], mul=1.0 / nrm)
        var = rowp.tile([1, TCn], F32, tag="rows", name="var")
        nc.scalar.mul(out=var[:], in_=ps1[:], mul=1.0 / nrm)
        msq = rowp.tile([1, TCn], F32, tag="lnt", name="msq")
        nc.vector.tensor_mul(out=msq[:], in0=mu[:], in1=mu[:])
        nc.vector.tensor_sub(out=var[:], in0=var[:], in1=msq[:])
        rho = rowp.tile([1, TCn], F32, tag="lnt", name="rho")
        nc.scalar.activation(out=var[:], in_=var[:], func=AF.Ln,
                             bias=c_eps1[:], scale=1.0)
        nc.scalar.activation(out=rho[:], in_=var[:], func=AF.Exp,
                             scale=-0.5)
        dma(out=tn["stat_d"][stat_row:stat_row + 1, tsl], in_=mu[:])
        dma(out=tn["stat_d"][stat_row + 1:stat_row + 2, tsl], in_=rho[:])

    def bcast_stats(stat_row, tsl):
        b_mu = sbp.tile([P, TCn], F32, tag="bmu", name="b_mu")
        b_rho = sbp.tile([P, TCn], F32, tag="brho", name="b_rho")
        dma(out=b_mu[:], in_=bass.AP(tensor=tn["stat_d"],
                                     offset=stat_row * g.t + tsl.start,
                                     ap=[[0, P], [1, TCn]]))
        dma(out=b_rho[:], in_=bass.AP(tensor=tn["stat_d"],
                                      offset=(stat_row + 1) * g.t + tsl.start,
                                      ap=[[0, P], [1, TCn]]))
        return b_mu, b_rho

    # ==================================================================
    # phase 1: per time-chunk sc: LN1 -> variants -> proj -> WKV -> GN
    #          -> Wo partials -> RS -> xmid -> AG
    # ==================================================================
    with contextlib.ExitStack() as p1:
        def pool1(name, bufs, space="SBUF"):
            return p1.enter_context(
                tc.tile_pool(name=name, bufs=bufs, space=space))

        pm = pool1("pm", 2, space="PSUM")        # matmul scratch (N<=512)
        pproj = pool1("pproj", 2, space="PSUM")  # projection banks
        pwa = pool1("pwa", 2, space="PSUM")      # wkv bank A (pkt|pA|pruk)
        pwb = pool1("pwb", 2, space="PSUM")      # wkv bank B (pO|pS)

        lnp = pool1("lnp", 2)       # LN1 streamed rows
        xnp = pool1("xn", 2)        # xn / xx per sc
        vp = pool1("vars", 2)       # variant tiles
        t0p = pool1("t0", 2)        # small scratch
        rk = pool1("rkvgw", 2)      # rT/kT/v/g per sc
        wtp = pool1("wt", 1)        # wT fp32 per sc
        prep = pool1("prep", 1)     # lam / E1 per sc
        wkvp = pool1("wkv", 2)      # small wkv scratch
        Sp = pool1("state", 2)      # wkv state
        gnp = pool1("gn", 2)        # groupnorm scratch
        ogp = pool1("og", 1)        # ogT per sc
        wlh = pool1("wlh", 2)       # streamed lhsT weights
        wvg = pool1("wvg", 2)       # streamed rhs weights (v/g)
        evp = pool1("evac", 2)      # cc1 evacs + xmid

        S_cur = Sp.tile([P, g.npair, g.hs], F32, tag="S")
        nc.vector.memset(S_cur, 0.0)
        xn_prev = [None]

        for sc in range(g.nsc):
            t0 = sc * TCn
            tsl = slice(t0, t0 + TCn)

            # ---- LN1 stats ----
            def x_chunk(j):
                xt = lnp.tile([P, TCn], F32R, tag="ldr", name="xt")
                dma(out=xt[:], in_=tn["xT"][bass.ts(j, P), tsl].bitcast(F32R))
                sq = lnp.tile([P, TCn], F32R, tag="sq", name="sq")
                nc.scalar.square(out=sq[:], in_=xt[:])
                return xt[:], sq[:]

            ln_stats_rows(pm, lnp, t0p, x_chunk, g.ct, tsl, g.c, EPS_LN, 0)
            b_mu, b_rho = bcast_stats(0, tsl)

            # ---- LN1 apply -> xn, xx (fp16) ----
            xn = xnp.tile([P, g.ct, TCn], F16, tag="xn", name="xn")
            xx = xnp.tile([P, g.ct, TCn], F16, tag="xx", name="xx", bufs=1)
            for j in range(g.ct):
                row = lnp.tile([P, TCn], F32R, tag="ldr", name="row")
                dma(out=row[:], in_=tn["xT"][bass.ts(j, P), tsl].bitcast(F32R))
                tsc = t0p.tile([P, TCn], F32, tag="lnt", name="tsc")
                nc.any.tensor_sub(out=tsc[:], in0=row[:].bitcast(F32),
                                  in1=b_mu[:])
                nc.vector.scalar_tensor_tensor(out=xn[:, j, :], in0=tsc[:],
                                               scalar=coef_col(8, j),
                                               in1=b_rho[:],
                                               op0=OP.mult, op1=OP.mult)
                nc.any.tensor_sub(out=xx[:, j, 1:TCn],
                                  in0=xn[:, j, 0:TCn - 1],
                                  in1=xn[:, j, 1:TCn])
                if sc == 0:
                    nc.any.tensor_scalar_mul(out=xx[:, j, 0:1],
                                             in0=xn[:, j, 0:1], scalar1=-1.0)
                else:
                    nc.any.tensor_sub(out=xx[:, j, 0:1],
                                      in0=xn_prev[0][:, j, TCn - 1:TCn],
                                      in1=xn[:, j, 0:1])
            xn_prev[0] = xn

            if tn["debug"]:
                dbg_dump(gnp, "dbg_xn", xn[:, :, :],
                         tn["dbg_xn"].ap().rearrange(
                             "(j p) t -> p j t", p=P)[:, :, tsl])

            if tn["phases"] == 1:
                fin = t0p.tile([P, TCn], F32, tag="fin", name="fin")
                for o in range(g.oct):
                    nc.vector.tensor_copy(out=fin[:], in_=xn[:, o, :])
                    dma(out=tn["yT"][bass.ts(o, P), tsl], in_=fin[:])
                continue

            # ---- xxx = tanh(vx @ mw1) ----
            vx = vp.tile([P, g.ct, TCn], F16, tag="var", name="vx")
            for j in range(g.ct):
                if j % 2 == 0:
                    nc.vector.scalar_tensor_tensor(
                        out=vx[:, j, :], in0=xx[:, j, :],
                        scalar=coef_col(0, j), in1=xn[:, j, :],
                        op0=OP.mult, op1=OP.add)
                else:
                    nc.gpsimd.tensor_scalar_mul(out=vx[:, j, :],
                                                in0=xx[:, j, :],
                                                scalar1=coef_col(0, j))
                    nc.gpsimd.tensor_add(out=vx[:, j, :], in0=vx[:, j, :],
                                         in1=xn[:, j, :])
            xxa = t0p.tile([P, TCn], F16, tag="xxa", name="xxa")
            pa = pm.tile([P, TCn], F32, tag="m", name="pa")
            for j in range(g.ct):
                nc.tensor.matmul(pa[:], c_mw1[:, j, 0:P], vx[:, j, :],
                                 start=(j == 0), stop=(j == g.ct - 1))
            nc.scalar.activation(out=xxa[:], in_=pa[:], func=AF.Tanh)
            xxb = t0p.tile([g.tm, TCn], F16, tag="xxb", name="xxb")
            pb = pm.tile([g.tm, TCn], F32, tag="m", name="pb")
            for j in range(g.ct):
                nc.tensor.matmul(pb[:], c_mw1[:, j, P:5 * g.tm], vx[:, j, :],
                                 start=(j == 0), stop=(j == g.ct - 1))
            nc.scalar.activation(out=xxb[:], in_=pb[:], func=AF.Tanh)

            def build_variant(f):
                var = vp.tile([P, g.ct, TCn], F16, tag="var", name="var")
                for j in range(g.ct):
                    mp = pm.tile([P, TCn], F32, tag="m", name="mp")
                    if f < 4:
                        nc.tensor.matmul(mp[:],
                                         c_mw2a[bass.ts(f, g.tm),
                                                bass.ts(j, P)],
                                         xxa[bass.ts(f, g.tm), :],
                                         start=True, stop=True,
                                         tile_position=(f * g.tm, 0))
                    else:
                        nc.tensor.matmul(mp[:], c_mw2b[:, bass.ts(j, P)],
                                         xxb[:], start=True, stop=True)
                    tt = t0p.tile([P, TCn], F16, tag="t0", name="tt")
                    if j % 2 == 0:
                        nc.scalar.activation(out=tt[:], in_=mp[:],
                                             func=AF.Identity,
                                             bias=coef_col(1 + f, j),
                                             scale=1.0)
                        nc.any.tensor_mul(out=var[:, j, :], in0=tt[:],
                                          in1=xx[:, j, :])
                    else:
                        nc.vector.scalar_tensor_tensor(
                            out=tt[:], in0=mp[:], scalar=coef_col(1 + f, j),
                            in1=xx[:, j, :], op0=OP.add, op1=OP.mult)
                        nc.any.tensor_copy(out=var[:, j, :], in_=tt[:])
                    nc.any.tensor_add(out=var[:, j, :], in0=var[:, j, :],
                                      in1=xn[:, j, :])
                return var

            rT = rk.tile([P, g.oct, TCn], F16, tag="rT", name="rT")
            kT = rk.tile([P, g.oct, TCn], F16, tag="kT", name="kT")
            wT = wtp.tile([P, g.oct, TCn], F32, tag="wT", name="wT")
            v_tm = rk.tile([P, g.nl, g.oc], F16, tag="v_tm", name="v_tm")
            g_tm = rk.tile([P, g.nl, g.oc], F16, tag="g_tm", name="g_tm", bufs=1)

            def proj_cm(var, wname, out_t, o, bias=None):
                wt = wlh.tile([P, g.ct, P], F16, tag="wl", name="wt")
                dma(out=wt[:], in_=tn[wname][o, :, :, :])
                po = pproj.tile([P, TCn], F32, tag="proj", name="po")
                for j in range(g.ct):
                    nc.tensor.matmul(po[:], wt[:, j, :], var[:, j, :],
                                     start=(j == 0), stop=(j == g.ct - 1))
                if bias is None:
                    nc.scalar.copy(out=out_t[:, o, :], in_=po[:])
                else:
                    nc.scalar.activation(out=out_t[:, o, :], in_=po[:],
                                         func=AF.Identity, bias=bias,
                                         scale=1.0)

            # --- w path ---
            var_w = build_variant(0)
            pw = pm.tile([g.td, TCn], F32, tag="m", name="pw")
            for j in range(g.ct):
                nc.tensor.matmul(pw[:], c_tdw1[:, j, :], var_w[:, j, :],
                                 start=(j == 0), stop=(j == g.ct - 1))
            th = t0p.tile([g.td, TCn], F16, tag="th", name="th")
            nc.scalar.activation(out=th[:], in_=pw[:], func=AF.Tanh)
            for o in range(g.oct):
                po = pproj.tile([P, TCn], F32, tag="proj", name="po")
                nc.tensor.matmul(po[:], c_tdw2[:, bass.ts(o, P)], th[:],
                                 start=True, stop=True)
                nc.scalar.activation(out=wT[:, o, :], in_=po[:],
                                     func=AF.Identity,
                                     bias=c_tdec[:, o:o + 1], scale=1.0)

            # --- k, v, r, g ---
            var_k = build_variant(1)
            for o in range(g.oct):
                proj_cm(var_k, "wkT", kT, o)
            var_v = build_variant(2)
            for half in range(2):
                pos = [pproj.tile([P, g.oc], F32, tag="proj", name=f"pos{i}")
                       for i in range(2)]
                for j in range(g.ct):
                    wt = wvg.tile([P, g.oc], F16, tag="wvg", name="wtv")
                    dma(out=wt[:], in_=tn["wvT"][:, j, :])
                    for i in range(2):
                        l = half * 2 + i
                        nc.tensor.matmul(pos[i][0:g.l, :],
                                         var_v[:, j, bass.ts(l, g.l)], wt[:],
                                         start=(j == 0),
                                         stop=(j == g.ct - 1))
                for i in range(2):
                    nc.scalar.copy(out=v_tm[0:g.l, half * 2 + i, :],
                                   in_=pos[i][0:g.l, :])
            var_r = build_variant(3)
            for o in range(g.oct):
                proj_cm(var_r, "wrT", rT, o)
            var_g = build_variant(4)
            for half in range(2):
                pos = [pproj.tile([P, g.oc], F32, tag="proj", name=f"pog{i}")
                       for i in range(2)]
                for j in range(g.ct):
                    wt = wvg.tile([P, g.oc], F16, tag="wvg", name="wtg")
                    dma(out=wt[:], in_=tn["wgT"][:, j, :])
                    for i in range(2):
                        l = half * 2 + i
                        nc.tensor.matmul(pos[i][0:g.l, :],
                                         var_g[:, j, bass.ts(l, g.l)], wt[:],
                                         start=(j == 0),
                                         stop=(j == g.ct - 1))
                for i in range(2):
                    l = half * 2 + i
                    tg = t0p.tile([P, g.oc], F16, tag="tg", name="tg")
                    nc.scalar.activation(out=tg[0:g.l, :],
                                         in_=pos[i][0:g.l, :],
                                         func=AF.Tanh, scale=0.5)
                    # g = 0.5*x*(1+tanh(x/2)); the 0.5 is folded into Wo
                    nc.vector.scalar_tensor_tensor(
                        out=g_tm[0:g.l, l, :], in0=tg[0:g.l, :],
                        scalar=1.0, in1=pos[i][0:g.l, :],
                        op0=OP.add, op1=OP.mult)

            if tn["debug"]:
                dbg_dump(gnp, "dbg_r", rT[:, :, :],
                         tn["dbg_r"].ap().rearrange(
                             "(o p) t -> p o t", p=P)[:, :, tsl])
                dbg_dump(gnp, "dbg_k", kT[:, :, :],
                         tn["dbg_k"].ap().rearrange(
                             "(o p) t -> p o t", p=P)[:, :, tsl])
                dbg_dump(gnp, "dbg_w", wT[:, :, :],
                         tn["dbg_w"].ap().rearrange(
                             "(o p) t -> p o t", p=P)[:, :, tsl], cast=False)
                for l in range(g.nl):
                    lsl2 = slice(t0 + l * g.l, t0 + (l + 1) * g.l)
                    dbg_dump(gnp, "dbg_v", v_tm[0:g.l, l, :],
                             tn["dbg_v"][lsl2, :])
                    dbg_dump(gnp, "dbg_g", g_tm[0:g.l, l, :],
                             tn["dbg_g"][lsl2, :])

            if tn["phases"] == 2:
                fin = t0p.tile([P, TCn], F32, tag="fin", name="fin")
                for o in range(g.oct):
                    nc.vector.tensor_copy(out=fin[:], in_=rT[:, o, :])
                    dma(out=tn["yT"][bass.ts(o, P), tsl], in_=fin[:])
                continue

            # ---- WKV prep (batched per sc) ----
            ex = prep.tile([P, g.oct, TCn], F32, tag="we", name="ex")
            nc.scalar.activation(out=ex[:], in_=wT[:], func=AF.Exp)
            lam = wtp.tile([P, g.oct, TCn], F32, tag="wT", name="lam")
            for p in range(g.npair):
                for l in range(g.nl):
                    lsl = slice(l * g.l, (l + 1) * g.l)
                    nc.vector.tensor_tensor_scan(
                        out=lam[:, p, lsl], data0=ex[:, p, lsl],
                        data1=c_neg1[:], initial=0.0,
                        op0=OP.subtract, op1=OP.mult)
            E1 = prep.tile([P, g.oct, TCn], F32, tag="we", name="E1")
            nc.scalar.activation(out=E1[:], in_=lam[:], func=AF.Exp)

            # ---- WKV chunks ----
            ogT = ogp.tile([P, g.oct, TCn], F16, tag="ogT", name="ogT")
            for l in range(g.nl):
                lsl = slice(l * g.l, (l + 1) * g.l)
                O_sb = gnp.tile([g.l, g.oct, P], F32, tag="O", name="O_sb")
                S_nxt = Sp.tile([P, g.npair, g.hs], F32, tag="S", name="S_n")
                for p in range(g.npair):
                    pl = E1[:, p, l * g.l + g.l - 1:(l + 1) * g.l]
                    e2 = wkvp.tile([P, g.l], F32, tag="e2", name="e2")
                    nc.scalar.activation(out=e2[:], in_=lam[:, p, lsl],
                                         func=AF.Exp, scale=-1.0)
                    kt = wkvp.tile([P, g.l], F32, tag="kt", name="kt")
                    nc.vector.tensor_mul(out=kt[:], in0=kT[:, p, lsl],
                                         in1=e2[:])
                    rt = wkvp.tile([P, g.l], F32, tag="rt", name="rt")
                    nc.gpsimd.tensor_mul(
                        out=rt[:, 1:g.l],
                        in0=rT[:, p, l * g.l:(l + 1) * g.l - 1],
                        in1=E1[:, p, l * g.l:(l + 1) * g.l - 1])
                    nc.gpsimd.tensor_copy(out=rt[:, 0:1],
                                          in_=rT[:, p, l * g.l:l * g.l + 1])
                    khat = wkvp.tile([P, g.l], F16, tag="khat", name="khat")
                    nc.gpsimd.tensor_scalar_mul(out=khat[:], in0=kt[:],
                                                scalar1=pl)
                    rukp = wkvp.tile([P, g.l], F16, tag="rukp", name="rukp")
                    nc.gpsimd.tensor_scalar_mul(out=rukp[:],
                                                in0=rT[:, p, lsl],
                                                scalar1=c_u[:, p:p + 1])
                    nc.gpsimd.tensor_mul(out=rukp[:], in0=rukp[:],
                                         in1=kT[:, p, lsl])
                    # bank A: pkt | pA0 | pA1 | pruk
                    bA = pwa.tile([P, 512], F32, tag="wa", name="bA")
                    bA16 = bA.bitcast(F16)
                    pkt = bA16[:, 0:P]
                    nc.tensor.transpose(pkt, khat[:, :], c_id[:, :])
                    ktm = wkvp.tile([g.l, P], F16, tag="ktm", name="ktm")
                    nc.vector.tensor_copy(out=ktm[:], in_=pkt)
                    pA = [bA[:, 64 + 128 * hh:64 + 128 * (hh + 1)]
                          for hh in range(2)]
                    for hh in range(2):
                        hsl = slice(hh * g.hs, (hh + 1) * g.hs)
                        nc.tensor.matmul(pA[hh], kt[hsl, :], rt[hsl, :],
                                         start=True, stop=True)
                    pruk = bA[:, 320:322]
                    nc.tensor.matmul(pruk, rukp[:, :], c_ones2[:, :],
                                     start=True, stop=True)
                    Am = wkvp.tile([g.l, 2 * g.l], F16, tag="Am", name="Am")
                    for hh in range(2):
                        nc.vector.tensor_mul(out=Am[:, bass.ts(hh, g.l)],
                                             in0=pA[hh], in1=c_mask[:, :])
                    ruk = wkvp.tile([g.l, 2], F32, tag="ruks", name="ruk")
                    nc.vector.tensor_copy(out=ruk[:], in_=pruk)
                    # bank B: pO0 | pO1 | pS
                    bB = pwb.tile([P, 512], F32, tag="wb", name="bB")
                    pO = [bB[:, 64 * hh:64 * (hh + 1)] for hh in range(2)]
                    pS = bB[:, 128:192]
                    for hh in range(2):
                        hsl = slice(hh * g.hs, (hh + 1) * g.hs)
                        csl = slice((2 * p + hh) * g.hs,
                                    (2 * p + hh + 1) * g.hs)
                        nc.tensor.matmul(pO[hh], Am[:, bass.ts(hh, g.l)],
                                         v_tm[0:g.l, l, csl],
                                         start=True, stop=False)
                        nc.tensor.matmul(pO[hh], rt[hsl, :],
                                         S_cur[hsl, p, :],
                                         start=False, stop=True)
                        nc.tensor.matmul(bB[hsl, 128:192],
                                         ktm[:, hsl], v_tm[0:g.l, l, csl],
                                         start=True, stop=True,
                                         skip_group_check=True)
                    for hh in range(2):
                        csl = slice((2 * p + hh) * g.hs,
                                    (2 * p + hh + 1) * g.hs)
                        nc.vector.scalar_tensor_tensor(
                            out=O_sb[:, p, slice(hh * g.hs, (hh + 1) * g.hs)],
                            in0=v_tm[0:g.l, l, csl],
                            scalar=ruk[:, hh:hh + 1],
                            in1=pO[hh],
                            op0=OP.mult, op1=OP.add)
                    tS = wkvp.tile([P, g.hs], F32, tag="tS", name="tS")
                    nc.vector.tensor_scalar_mul(out=tS[:],
                                                in0=S_cur[:, p, :],
                                                scalar1=pl)
                    nc.vector.tensor_add(out=S_nxt[:, p, :], in0=tS[:],
                                         in1=pS)
                S_cur = S_nxt

                # ---- GroupNorm + *g + transpose ----
                Ov = O_sb[:, :, :].rearrange("t a b -> t (a b)").rearrange(
                    "t (h d) -> t h d", d=g.hs)
                Of = O_sb[:, :, :].rearrange("t a b -> t (a b)")
                if tn["debug"]:
                    lsl2 = slice(t0 + l * g.l, t0 + (l + 1) * g.l)
                    dbg_dump(gnp, "dbg_o", Of, tn["dbg_o"][lsl2, :],
                             cast=False)
                sums = gnp.tile([g.l, g.nh], F32, tag="sums", name="sums")
                nc.vector.reduce_sum(out=sums[:], in_=Ov, axis=AX.X)
                osq = gnp.tile([g.l, g.oc], F32, tag="osq", name="osq", bufs=1)
                nc.scalar.square(out=osq[:], in_=Of)
                sqs = gnp.tile([g.l, g.nh], F32, tag="sqs", name="sqs")
                nc.vector.reduce_sum(out=sqs[:],
                                     in_=osq.rearrange("t (h d) -> t h d",
                                                       d=g.hs), axis=AX.X)
                mean = gnp.tile([g.l, g.nh], F32, tag="mean", name="mean")
                nc.scalar.mul(out=mean[:], in_=sums[:], mul=1.0 / g.hs)
                var_ = gnp.tile([g.l, g.nh], F32, tag="var", name="var_")
                msq = gnp.tile([g.l, g.nh], F32, tag="msq", name="msq")
                nc.vector.tensor_mul(out=msq[:], in0=mean[:], in1=mean[:])
                nc.vector.scalar_tensor_tensor(out=var_[:], in0=sqs[:],
                                               scalar=1.0 / g.hs,
                                               in1=msq[:],
                                               op0=OP.mult, op1=OP.subtract)
                rstd = gnp.tile([g.l, g.nh], F32, tag="rstd", name="rstd")
                nc.scalar.activation(out=var_[:], in_=var_[:], func=AF.Ln,
                                     bias=c_epsg[0:g.l, :], scale=1.0)
                nc.scalar.activation(out=rstd[:], in_=var_[:], func=AF.Exp,
                                     scale=-0.5)
                og16 = gnp.tile([g.l, g.oc], F16, tag="og16", name="og16", bufs=1)
                ogv = og16.rearrange("t (h d) -> t h d", d=g.hs)
                for h in range(g.nh):
                    nc.vector.tensor_scalar(out=ogv[:, h, :],
                                            in0=Ov[:, h, :],
                                            scalar1=mean[:, h:h + 1],
                                            scalar2=rstd[:, h:h + 1],
                                            op0=OP.subtract, op1=OP.mult)
                nc.any.tensor_mul(out=og16[:], in0=og16[:],
                                  in1=g_tm[0:g.l, l, :])
                if tn["debug"]:
                    lsl2 = slice(t0 + l * g.l, t0 + (l + 1) * g.l)
                    dbg_dump(gnp, "dbg_og", og16[:, :],
                             tn["dbg_og"][lsl2, :])
                for o in range(g.oct):
                    bB = pwb.tile([P, 512], F32, tag="wb", name="bT")
                    pt = bB.bitcast(F16)[:, 0:g.l]
                    nc.tensor.transpose(pt, og16[:, bass.ts(o, P)],
                                        c_id[0:g.l, 0:g.l])
                    nc.vector.tensor_copy(out=ogT[:, o, lsl], in_=pt)

            if tn["debug"]:
                dbg_dump(gnp, "dbg_ogT", ogT[:, :, :],
                         tn["dbg_ogT"].ap().rearrange(
                             "(o p) t -> p o t", p=P)[:, :, tsl])

            # ---- Wo partials -> cc1_in (fp16) ----
            for ot in range(g.ct):
                wt = wlh.tile([P, g.oct, P], F16, tag="wl", name="wto")
                dma(out=wt[:], in_=tn["woT"][ot, :, :, :])
                po = pproj.tile([P, TCn], F32, tag="proj", name="po")
                for p in range(g.oct):
                    nc.tensor.matmul(po[:], wt[:, p, :], ogT[:, p, :],
                                     start=(p == 0), stop=(p == g.oct - 1))
                ev = evp.tile([P, TCn], F16, tag="ev", name="ev")
                if ot % 2 == 0:
                    nc.scalar.copy(out=ev[:], in_=po[:])
                else:
                    nc.vector.tensor_copy(out=ev[:], in_=po[:])
                dma(out=tn["cc1_in"][sc, bass.ts(ot, P), :], in_=ev[:])

            if tn["phases"] >= 4:
                # ---- ReduceScatter of Wo partials for this chunk ----
                nc.gpsimd.collective_compute(
                    "ReduceScatter", OP.add, replica_groups=tn["groups"],
                    ins=[tn["cc1_in"][sc, :, :].opt()],
                    outs=[tn["cc1_out"][sc, :, :].opt()])
                # ---- xmid16 = x_sl + att_sl -> DRAM ----
                for o in range(g.oct):
                    att = evp.tile([P, TCn], F16, tag="att", name="att", bufs=1)
                    dma(out=att[:], in_=tn["cc1_out"][sc, bass.ts(o, P), :])
                    xsl = evp.tile([P, TCn], F32, tag="xsl", name="xsl", bufs=1)
                    dma(out=xsl[:], in_=tn["x_sl"][bass.ts(o, P), tsl])
                    xm = evp.tile([P, TCn], F16, tag="xm", name="xm", bufs=1)
                    nc.any.tensor_add(out=xm[:], in0=att[:], in1=xsl[:])
                    dma(out=tn["xmid_d"][sc, bass.ts(o, P), :], in_=xm[:])
                # ---- AllGather xmid for this chunk ----
                nc.gpsimd.collective_compute(
                    "AllGather", OP.bypass, replica_groups=tn["groups"],
                    ins=[tn["xmid_d"][sc, :, :].opt()],
                    outs=[tn["xag_d"][sc, :, :].opt()])

    if tn["phases"] in (1, 2):
        return
    if tn["phases"] == 3:
        fin2 = ctx.enter_context(tc.tile_pool(name="fin2", bufs=2))
        for sc in range(g.nsc):
            for o in range(g.oct):
                tt_ = fin2.tile([P, g.tc], F32, tag="f", name="tt_")
                dma(out=tt_[:], in_=tn["cc1_out"][sc, bass.ts(o, P), :])
                dma(out=tn["yT"][bass.ts(o, P),
                                 sc * g.tc:(sc + 1) * g.tc], in_=tt_[:])
        return

    # ==================================================================
    # phase 2: per half: LN2 -> FFN -> RS -> combine
    # ==================================================================
    with contextlib.ExitStack() as p2:
        def pool2(name, bufs, space="SBUF"):
            return p2.enter_context(
                tc.tile_pool(name=name, bufs=bufs, space=space))

        pmf = pool2("pmf", 2, space="PSUM")
        pf = pool2("pf", 3, space="PSUM")
        xagp = pool2("xag", 2)
        ln2p = pool2("ln2", 2)
        xn2p = pool2("xn2", 2)
        v2p = pool2("v2", 1)
        kfp = pool2("kf", 1)
        wlh2 = pool2("wlh2", 3)
        t2p = pool2("t2", 3)
        ev2p = pool2("ev2", 3)
        sigp = pool2("sig", 2)

        xn2_prev = [None]
        for sc in range(g.nsc):
            t0 = sc * TCn
            tsl = slice(t0, t0 + TCn)
            # ---- load gathered xmid (fp16) ----
            xag = xagp.tile([P, g.ct, TCn], F16, tag="xag", name="xag")
            for j in range(g.ct):
                dma(out=xag[:, j, :], in_=tn["xag_d"][sc, bass.ts(j, P), :])

            # ---- LN2 stats ----
            ps0 = pmf.tile([1, TCn], F32, tag="m", name="ps0")
            ps1 = pmf.tile([1, TCn], F32, tag="m", name="ps1")
            for j in range(g.ct):
                sq = ln2p.tile([P, TCn], F16, tag="sq2", name="sq2")
                nc.scalar.square(out=sq[:], in_=xag[:, j, :])
                nc.tensor.matmul(ps0[:], c_ones16, xag[:, j, :],
                                 start=(j == 0), stop=(j == g.ct - 1))
                nc.tensor.matmul(ps1[:], c_ones16, sq[:],
                                 start=(j == 0), stop=(j == g.ct - 1))
            mu = t2p.tile([1, TCn], F32, tag="rows", name="mu2")
            nc.scalar.mul(out=mu[:], in_=ps0[:], mul=1.0 / g.c)
            var = t2p.tile([1, TCn], F32, tag="rows", name="var2")
            nc.scalar.mul(out=var[:], in_=ps1[:], mul=1.0 / g.c)
            msq = t2p.tile([1, TCn], F32, tag="rows", name="msq2")
            nc.vector.tensor_mul(out=msq[:], in0=mu[:], in1=mu[:])
            nc.vector.tensor_sub(out=var[:], in0=var[:], in1=msq[:])
            rho = t2p.tile([1, TCn], F32, tag="rows", name="rho2")
            nc.scalar.activation(out=var[:], in_=var[:], func=AF.Ln,
                                 bias=c_eps1[:], scale=1.0)
            nc.scalar.activation(out=rho[:], in_=var[:], func=AF.Exp,
                                 scale=-0.5)
            dma(out=tn["stat_d"][4:5, tsl], in_=mu[:])
            dma(out=tn["stat_d"][5:6, tsl], in_=rho[:])
            b_mu, b_rho = bcast_stats(4, tsl)

            # ---- LN2 apply -> xn2 (fp16); no xx2 buffer ----
            xn2 = xn2p.tile([P, g.ct, TCn], F16, tag="xn2", name="xn2")
            for j in range(g.ct):
                tsc = t2p.tile([P, TCn], F32, tag="lnt2", name="tsc2")
                nc.any.tensor_sub(out=tsc[:], in0=xag[:, j, :], in1=b_mu[:])
                nc.vector.scalar_tensor_tensor(out=xn2[:, j, :], in0=tsc[:],
                                               scalar=coef_col(9, j),
                                               in1=b_rho[:],
                                               op0=OP.mult, op1=OP.mult)
            if tn["debug"]:
                dbg_dump(t2p, "dbg_xn2", xn2[:, :, :],
                         tn["dbg_xn2"].ap().rearrange(
                             "(j p) t -> p j t", p=P)[:, :, tsl])
            if tn["phases"] == 5:
                fin = t2p.tile([P, TCn], F32, tag="fin", name="fin")
                for o in range(g.oct):
                    nc.vector.tensor_copy(out=fin[:], in_=xn2[:, o, :])
                    dma(out=tn["yT"][bass.ts(o, P), tsl], in_=fin[:])
                xn2_prev[0] = xn2
                continue

            # ---- FFN variant vk2 = (1-c)*xn2 + c*shift(xn2) ----
            def ffn_variant(row, c1_row, name):
                vv = v2p.tile([P, g.ct, TCn], F16, tag="v2", name=name)
                for j in range(g.ct):
                    if j % 2 == 0:
                        nc.gpsimd.tensor_scalar_mul(
                            out=vv[:, j, 1:TCn], in0=xn2[:, j, 1:TCn],
                            scalar1=c_c1m[:, c1_row, j:j + 1])
                        nc.vector.scalar_tensor_tensor(
                            out=vv[:, j, 1:TCn], in0=xn2[:, j, 0:TCn - 1],
                            scalar=coef_col(row, j), in1=vv[:, j, 1:TCn],
                            op0=OP.mult, op1=OP.add)
                    else:
                        tv = t2p.tile([P, TCn], F16, tag="tv", name="tv")
                        nc.gpsimd.tensor_scalar_mul(
                            out=tv[:, 1:TCn], in0=xn2[:, j, 0:TCn - 1],
                            scalar1=coef_col(row, j))
                        nc.gpsimd.tensor_scalar_mul(
                            out=vv[:, j, 1:TCn], in0=xn2[:, j, 1:TCn],
                            scalar1=c_c1m[:, c1_row, j:j + 1])
                        nc.gpsimd.tensor_add(out=vv[:, j, 1:TCn],
                                             in0=vv[:, j, 1:TCn],
                                             in1=tv[:, 1:TCn])
                    # boundary col t0
                    nc.vector.tensor_scalar_mul(
                        out=vv[:, j, 0:1], in0=xn2[:, j, 0:1],
                        scalar1=c_c1m[:, c1_row, j:j + 1])
                    if sc > 0:
                        nc.vector.scalar_tensor_tensor(
                            out=vv[:, j, 0:1],
                            in0=xn2_prev[0][:, j, TCn - 1:TCn],
                            scalar=coef_col(row, j), in1=vv[:, j, 0:1],
                            op0=OP.mult, op1=OP.add)
                return vv

            vk2 = ffn_variant(6, 0, "vk2")
            kf = kfp.tile([P, g.fct, TCn], F16, tag="kf", name="kf")
            for ft in range(g.fct):
                wt = wlh2.tile([P, g.ct, P], F16, tag="wl2", name="wt2")
                dma(out=wt[:], in_=tn["wkfT"][ft, :, :, :])
                po = pf.tile([P, TCn], F32, tag="proj", name="po2")
                for j in range(g.ct):
                    nc.tensor.matmul(po[:], wt[:, j, :], vk2[:, j, :],
                                     start=(j == 0), stop=(j == g.ct - 1))
                tr = t2p.tile([P, TCn], F16, tag="relu", name="tr")
                nc.scalar.activation(out=tr[:], in_=po[:], func=AF.Relu)
                nc.any.tensor_mul(out=kf[:, ft, :], in0=tr[:], in1=tr[:])
            if tn["debug"]:
                dbg_dump(t2p, "dbg_kf", kf[:, :, :],
                         tn["dbg_kf"].ap().rearrange(
                             "(f p) t -> p f t", p=P)[:, :, tsl])
            for ot in range(g.ct):
                wt = wlh2.tile([P, g.fct, P], F16, tag="wl2", name="wt3")
                dma(out=wt[:], in_=tn["wvfT"][ot, :, :, :])
                po = pf.tile([P, TCn], F32, tag="proj", name="po3")
                for ft in range(g.fct):
                    nc.tensor.matmul(po[:], wt[:, ft, :], kf[:, ft, :],
                                     start=(ft == 0), stop=(ft == g.fct - 1))
                ev = ev2p.tile([P, TCn], F16, tag="ev2", name="ev2")
                if ot % 2 == 0:
                    nc.scalar.copy(out=ev[:], in_=po[:])
                else:
                    nc.vector.tensor_copy(out=ev[:], in_=po[:])
                dma(out=tn["cc2_in"][sc, bass.ts(ot, P), :], in_=ev[:])
            vr2 = ffn_variant(7, 1, "vr2")
            sig = sigp.tile([P, g.oct, TCn], F16, tag="sig", name="sig")
            for o in range(g.oct):
                wt = wlh2.tile([P, g.ct, P], F16, tag="wl2", name="wt4")
                dma(out=wt[:], in_=tn["wrfT"][o, :, :, :])
                po = pf.tile([P, TCn], F32, tag="proj", name="po4")
                for j in range(g.ct):
                    nc.tensor.matmul(po[:], wt[:, j, :], vr2[:, j, :],
                                     start=(j == 0), stop=(j == g.ct - 1))
                # sigma(x) = 0.5*(1+tanh(x/2)); 0.5 folded into Wv_ffn
                nc.scalar.activation(out=sig[:, o, :], in_=po[:],
                                     func=AF.Tanh, scale=0.5)
            xn2_prev[0] = xn2

            # ---- RS of FFN value partials for this half ----
            nc.gpsimd.collective_compute(
                "ReduceScatter", OP.add, replica_groups=tn["groups"],
                ins=[tn["cc2_in"][sc, :, :].opt()],
                outs=[tn["cc2_out"][sc, :, :].opt()])

            # ---- final: y = xmid + (1+s)*kv' ----
            for o in range(g.oct):
                kv = ev2p.tile([P, TCn], F16, tag="kvl", name="kv")
                dma(out=kv[:], in_=tn["cc2_out"][sc, bass.ts(o, P), :])
                xm = ev2p.tile([P, TCn], F16, tag="xml", name="xm2")
                dma(out=xm[:], in_=tn["xmid_d"][sc, bass.ts(o, P), :])
                gate = ev2p.tile([P, TCn], F32, tag="gat", name="gate")
                nc.vector.scalar_tensor_tensor(out=gate[:],
                                               in0=sig[:, o, :],
                                               scalar=1.0, in1=kv[:],
                                               op0=OP.add, op1=OP.mult)
                yv = ev2p.tile([P, TCn], F32, tag="yv", name="yv")
                nc.any.tensor_add(out=yv[:], in0=gate[:], in1=xm[:])
                dma(out=tn["yT"][bass.ts(o, P), tsl], in_=yv[:])


# ---------------------------------------------------------------------------
# host-side sharding / gather
# ---------------------------------------------------------------------------

def shard_inputs(g: CFG, inputs):
    """inputs: full setup_inputs() dict (numpy).  Returns list of per-core
    input maps."""
    f16 = np.float16
    f32 = np.float32

    def a(x):
        return np.ascontiguousarray(x)

    x = inputs["x"].astype(f32)
    assert not np.any(inputs["ln1_b"]) and not np.any(inputs["ln2_b"]), \
        "kernel assumes zero LN bias"
    assert not np.any(inputs["lnx_b"]), "kernel assumes zero lnx_b"

    def lhsT_bank(wT, oct_, ct_):
        # wT: (c_in, c_out_local) -> (oct, 128kin, ct, 128m)
        cin, cout = wT.shape
        r = wT.reshape(ct_, P, oct_, P)        # (j, kin, o, m)
        return a(r.transpose(2, 1, 0, 3).astype(f16))

    maps = []
    for core in range(g.n_cores):
        b = core // g.tpg
        r = core % g.tpg
        osl = slice(r * g.oc, (r + 1) * g.oc)
        fsl = slice(r * g.fc, (r + 1) * g.fc)
        m = {}
        m["xT"] = a(x[b].T)
        m["x_sl"] = a(x[b].T[osl])
        m["wrT"] = lhsT_bank(inputs["Wr"].T[:, osl].astype(f32), g.oct, g.ct)
        m["wkT"] = lhsT_bank(inputs["Wk"].T[:, osl].astype(f32), g.oct, g.ct)
        # woT: rows = local og channels, cols = full C; fold lnx_w and the
        # 0.5 from the tanh form of silu
        woT = (0.5 * inputs["lnx_w"][osl, None] *
               inputs["Wo"].T[osl, :]).astype(f32)
        rr = woT.reshape(g.oct, P, g.ct, P)    # (p, kin, ot, m)
        m["woT"] = a(rr.transpose(2, 1, 0, 3).astype(f16))
        m["wkfT"] = lhsT_bank(inputs["Wk_ffn"].T[:, fsl].astype(f32),
                              g.fct, g.ct)
        # fold 0.5 from the tanh form of sigmoid into Wv_ffn
        rr = (0.5 * inputs["Wv_ffn"].T[fsl, :]).astype(f32).reshape(
            g.fct, P, g.ct, P)
        m["wvfT"] = a(rr.transpose(2, 1, 0, 3).astype(f16))
        m["wrfT"] = lhsT_bank(inputs["Wr_ffn"].T[:, osl].astype(f32),
                              g.oct, g.ct)
        m["wvT"] = a(inputs["Wv"].T[:, osl].astype(f32).reshape(
            g.ct, P, g.oc).transpose(1, 0, 2).astype(f16))
        m["wgT"] = a(inputs["Wg"].T[:, osl].astype(f32).reshape(
            g.ct, P, g.oc).transpose(1, 0, 2).astype(f16))
        m["tdw1"] = a(inputs["time_decay_w1"].astype(f32).reshape(
            g.ct, P, g.td).transpose(1, 0, 2).astype(f16))
        m["tdw2"] = a(inputs["time_decay_w2"][:, osl].astype(f16))
        m["mw1"] = a(inputs["time_maa_w1"].astype(f32).reshape(
            g.ct, P, 5 * g.tm).transpose(1, 0, 2).astype(f16))
        mw2 = inputs["time_maa_w2"].astype(f32)   # (5, tm, c)
        m["mw2a"] = a(mw2[:4].reshape(4 * g.tm, g.c).astype(f16))
        m["mw2b"] = a(mw2[4].astype(f16))
        coef = np.zeros((P, 10, g.ct), f32)
        for i, nm in enumerate(["time_maa_x", "time_maa_w", "time_maa_k",
                                "time_maa_v", "time_maa_r", "time_maa_g",
                                "ffn_maa_k", "ffn_maa_r", "ln1_w", "ln2_w"]):
            coef[:, i, :] = inputs[nm].astype(f32).reshape(g.ct, P).T
        m["coef"] = a(coef)
        m["tdec"] = a(inputs["time_decay"].astype(f32)[osl].reshape(
            g.oct, P).T)
        u = inputs["time_faaaa"].astype(f32).reshape(-1)[
            r * g.oc:(r + 1) * g.oc]
        m["u"] = a(u.reshape(g.npair, P).T)
        m["mask_su"] = a(np.triu(np.ones((g.l, g.l), f32), 1))
        m["ident"] = a(np.eye(P, dtype=f16))
        m["ones_r"] = a(np.ones((P, 1), f32))
        ones2 = np.zeros((P, 2), f16)
        ones2[0:64, 0] = 1.0
        ones2[64:128, 1] = 1.0
        m["ones2"] = a(ones2)
        maps.append(m)
    return maps


def assemble(g: CFG, results):
    out = np.empty((g.b, g.t, g.c), np.float32)
    for core, res in enumerate(results):
        b = core // g.tpg
        r = core % g.tpg
        out[b, :, r * g.oc:(r + 1) * g.oc] = res["yT"].T
    return out


# ---------------------------------------------------------------------------
# public entry point
# ---------------------------------------------------------------------------

@functools.lru_cache(maxsize=1)
def _get_nc():
    nc = build_nc(FULL)
    nc.compile()
    return nc


def kernel(**inputs):
    from concourse.bass_utils import run_bass_kernel_spmd
    g = FULL
    nc = _get_nc()
    in_maps = shard_inputs(g, {k: np.asarray(v) for k, v in inputs.items()})
    res = run_bass_kernel_spmd(nc, in_maps, core_ids=list(range(g.n_cores)))
    return assemble(g, res.results)
